# revision 9
# baseline (speedup 1.0000x reference)
"""Hypergraph 2-hop message passing (gnn_message_passing) on 8 trn2 cores.

Pipeline: x0 = feats@W+b -> y1 = v2e-mean(x0) -> x1 = e2v-mean(y1)
          -> y2 = v2e-mean(x1) -> x2 = e2v-mean(y2) -> softmax(x2)

Sharding: vertices and edges row-sharded across 8 cores. Each segment-mean
stage partitions incidence pairs by destination shard; sources are fetched
with batched indirect DMA (row gather, 32 tiles per SWDGE op) from an
AllGather'd full table held in Shared HBM. Segment sums are computed with
one-hot selection matmuls accumulating in PSUM; a ones-column appended to
every table row yields the denominator in the same matmul. All tables and
matmul operands are bf16 (f32 PSUM accumulation).
"""
import math
import os
import numpy as np
import ml_dtypes

# Persistent XLA compilation cache: repeat calls (and repeat processes) skip
# recompiling the unchanged program. Must be set before jax initializes.
os.environ.setdefault("JAX_COMPILATION_CACHE_DIR", "/tmp/jax_cache_kernel")

BF16 = ml_dtypes.bfloat16
_SHARED_AG = os.environ.get("K_SHARED", "1") == "1"

N = 200_000
E = 50_000
NNZ = 2_000_000
F_IN = 256
D = 128
DW = D + 1                 # feature row + ones column (denominator)
NC = 8
P = 128
KB = int(os.environ.get("K_KB", "1"))  # tiles per indirect gather (HW rejects >1)

V_SH = N // NC             # 25000
E_SH = E // NC             # 6250
V_BLK = math.ceil(V_SH / P)    # 196
E_BLK = math.ceil(E_SH / P)    # 49
V_PAD = V_BLK * P          # 25088
E_PAD = E_BLK * P          # 6272


def _build_stage(dst, src_rows, w, n_dst_sh, n_blk):
    """Partition pairs by destination shard, sort by destination, pad each
    128-destination block to a common (max-over-cores) tile count.

    dst: global destination ids [NNZ]; src_rows: padded-table row ids [NNZ]
    Returns per-core [128, T] arrays (idx int32, lid bf16, w bf16), T, and
    per-block tile counts (shared across cores).
    """
    core_of = dst // n_dst_sh
    loc = dst % n_dst_sh
    per_core = []
    counts = np.zeros((NC, n_blk), np.int64)
    for k in range(NC):
        m = core_of == k
        lo = loc[m]
        order = np.argsort(lo, kind="stable")
        lo = lo[order]
        sr = src_rows[m][order]
        wk = w[m][order]
        blk = lo // P
        counts[k] = np.bincount(blk, minlength=n_blk)
        per_core.append((lo, sr, wk))
    tiles = np.maximum(np.ceil(counts / P).astype(np.int64).max(axis=0), 1)  # [n_blk]
    T = int(tiles.sum())
    starts = np.zeros(n_blk + 1, np.int64)
    starts[1:] = np.cumsum(tiles * P)
    idx_all, lid_all, w_all = [], [], []
    for k in range(NC):
        lo, sr, wk = per_core[k]
        idx = np.zeros(T * P, np.int32)
        lid = np.zeros(T * P, np.float32)
        ww = np.zeros(T * P, np.float32)
        bstart = np.zeros(n_blk + 1, np.int64)
        bstart[1:] = np.cumsum(counts[k])
        for b in range(n_blk):
            s, e = bstart[b], bstart[b + 1]
            o = starts[b]
            idx[o:o + (e - s)] = sr[s:e]
            lid[o:o + (e - s)] = (lo[s:e] - b * P).astype(np.float32)
            ww[o:o + (e - s)] = wk[s:e]
        idx_all.append(np.ascontiguousarray(idx.reshape(T, P).T))
        lid_all.append(np.ascontiguousarray(lid.reshape(T, P).T.astype(BF16)))
        w_all.append(np.ascontiguousarray(ww.reshape(T, P).T.astype(BF16)))
    return idx_all, lid_all, w_all, T, [int(t) for t in tiles]


def _pad_rows_v(v):
    return (v // V_SH) * V_PAD + (v % V_SH)


def _pad_rows_e(e):
    return (e // E_SH) * E_PAD + (e % E_SH)


def _build_and_run(inputs, trace=False):
    from concourse import bacc, bass, mybir, tile
    from concourse.bass_utils import run_bass_kernel_spmd

    feats = np.asarray(inputs["feats"], np.float32)
    W = np.asarray(inputs["W"], np.float32)
    b = np.asarray(inputs["b"], np.float32)
    pair_v = np.asarray(inputs["pair_v"], np.int32)
    pair_e = np.asarray(inputs["pair_e"], np.int32)
    v2e_w = np.asarray(inputs["v2e_weight"], np.float32)
    e2v_w = np.asarray(inputs["e2v_weight"], np.float32)

    # ---------------- host-side index prep ----------------
    src_x = _pad_rows_v(pair_v)
    src_y = _pad_rows_e(pair_e)
    # stage A: v2e (edge destinations), used for hops 1 and 2
    stA = _build_stage(pair_e.astype(np.int64), src_x, v2e_w, E_SH, E_BLK)
    # stage B: e2v (vertex destinations), used for hops 1 and 2
    stB = _build_stage(pair_v.astype(np.int64), src_y, e2v_w, V_SH, V_BLK)
    TA, tilesA = stA[3], stA[4]
    TB, tilesB = stB[3], stB[4]

    # featsT per core, padded, fp8 (cast to bf16 on device during DMA load)
    FP8 = ml_dtypes.float8_e4m3
    featsT = []
    for k in range(NC):
        sh = np.zeros((V_PAD, F_IN), np.float32)
        sh[:V_SH] = feats[k * V_SH:(k + 1) * V_SH]
        featsT.append(np.ascontiguousarray(sh.T.astype(FP8)))
    Wb = np.ascontiguousarray(W.astype(BF16))
    b_mat = np.broadcast_to(b[None, :], (P, D)).copy().astype(np.float32)
    iota = np.broadcast_to(np.arange(P, dtype=np.float32)[None, :], (P, P)).astype(BF16).copy()

    # ---------------- build program ----------------
    f32 = mybir.dt.float32
    bf16 = mybir.dt.bfloat16
    i32 = mybir.dt.int32
    fp8 = mybir.dt.float8e4
    nc = bacc.Bacc("TRN2", target_bir_lowering=False, debug=False, num_devices=NC)
    p_ftT = nc.declare_dram_parameter("featsT", [F_IN, V_PAD], fp8, isOutput=False)
    p_W = nc.declare_dram_parameter("W", [F_IN, D], bf16, isOutput=False)
    p_b = nc.declare_dram_parameter("b_mat", [P, D], f32, isOutput=False)
    p_iota = nc.declare_dram_parameter("iota", [P, P], bf16, isOutput=False)
    p_idxA = nc.declare_dram_parameter("idxA", [P, TA], i32, isOutput=False)
    p_lidA = nc.declare_dram_parameter("lidA", [P, TA], bf16, isOutput=False)
    p_wA = nc.declare_dram_parameter("wA", [P, TA], bf16, isOutput=False)
    p_idxB = nc.declare_dram_parameter("idxB", [P, TB], i32, isOutput=False)
    p_lidB = nc.declare_dram_parameter("lidB", [P, TB], bf16, isOutput=False)
    p_wB = nc.declare_dram_parameter("wB", [P, TB], bf16, isOutput=False)
    p_out = nc.declare_dram_parameter("out", [V_PAD, D], bf16, isOutput=True)

    x0_sh = nc.dram_tensor("x0_sh", [V_PAD, DW], bf16)
    x0_full = nc.dram_tensor("x0_full", [NC * V_PAD, DW], bf16, addr_space="Shared" if _SHARED_AG else "Local")
    y1_sh = nc.dram_tensor("y1_sh", [E_PAD, DW], bf16)
    y1_full = nc.dram_tensor("y1_full", [NC * E_PAD, DW], bf16, addr_space="Shared" if _SHARED_AG else "Local")
    x1_sh = nc.dram_tensor("x1_sh", [V_PAD, DW], bf16)
    x1_full = nc.dram_tensor("x1_full", [NC * V_PAD, DW], bf16, addr_space="Shared" if _SHARED_AG else "Local")
    y2_sh = nc.dram_tensor("y2_sh", [E_PAD, DW], bf16)
    y2_full = nc.dram_tensor("y2_full", [NC * E_PAD, DW], bf16, addr_space="Shared" if _SHARED_AG else "Local")

    rg = [list(range(NC))]
    with tile.TileContext(nc) as tc:
        with tc.tile_pool(name="const", bufs=1) as cpool, \
             tc.tile_pool(name="tabs", bufs=1) as tpool, \
             tc.tile_pool(name="fstream", bufs=4) as fpool, \
             tc.tile_pool(name="gath", bufs=4) as gpool, \
             tc.tile_pool(name="sel", bufs=8) as selpool, \
             tc.tile_pool(name="fin", bufs=4) as wpool, \
             tc.tile_pool(name="outp", bufs=4) as opool, \
             tc.tile_pool(name="psum", bufs=6, space="PSUM") as ppool:

            t_W = cpool.tile([P, 2, D], bf16, tag="wt")
            nc.sync.dma_start(out=t_W[:, 0, :], in_=p_W[0:128, :])
            nc.sync.dma_start(out=t_W[:, 1, :], in_=p_W[128:256, :])
            t_b = cpool.tile([P, D], f32, tag="bmat")
            nc.sync.dma_start(out=t_b[:], in_=p_b[:])
            t_iota = cpool.tile([P, P], bf16, tag="iota")
            nc.sync.dma_start(out=t_iota[:], in_=p_iota[:])

            t_idxA = tpool.tile([P, TA], i32, tag="idxA")
            t_lidA = tpool.tile([P, TA], bf16, tag="lidA")
            t_wA = tpool.tile([P, TA], bf16, tag="wA")
            nc.sync.dma_start(out=t_idxA[:], in_=p_idxA[:])
            nc.sync.dma_start(out=t_lidA[:], in_=p_lidA[:])
            nc.sync.dma_start(out=t_wA[:], in_=p_wA[:])
            t_idxB = tpool.tile([P, TB], i32, tag="idxB")
            t_lidB = tpool.tile([P, TB], bf16, tag="lidB")
            t_wB = tpool.tile([P, TB], bf16, tag="wB")
            nc.sync.dma_start(out=t_idxB[:], in_=p_idxB[:])
            nc.sync.dma_start(out=t_lidB[:], in_=p_lidB[:])
            nc.sync.dma_start(out=t_wB[:], in_=p_wB[:])

            # ---- stage 0: x0 = feats @ W + b (featsT pre-transposed) ----
            for rt in range(V_BLK):
                ft = fpool.tile([P, 2, P], bf16, tag="ft")
                nc.gpsimd.dma_start(out=ft[:, 0, :], in_=p_ftT[0:128, rt * P:(rt + 1) * P])
                nc.gpsimd.dma_start(out=ft[:, 1, :], in_=p_ftT[128:256, rt * P:(rt + 1) * P])
                ps = ppool.tile([P, DW], f32, tag="acc", name=f"ps0_{rt}")
                nc.tensor.matmul(out=ps[:, 0:D], lhsT=ft[:, 0, :], rhs=t_W[:, 0, :], start=True, stop=False)
                nc.tensor.matmul(out=ps[:, 0:D], lhsT=ft[:, 1, :], rhs=t_W[:, 1, :], start=False, stop=True)
                ob = opool.tile([P, DW], bf16, tag="x0o")
                nc.vector.tensor_tensor(out=ob[:, 0:D], in0=ps[:, 0:D], in1=t_b[:], op=mybir.AluOpType.add)
                nc.vector.memset(ob[:, D:DW], 1.0)
                nc.sync.dma_start(out=x0_sh[rt * P:(rt + 1) * P, :], in_=ob[:])
            nc.gpsimd.collective_compute("AllGather", mybir.AluOpType.bypass,
                                         replica_groups=rg, ins=[x0_sh[:]], outs=[x0_full[:]])

            # ---- segment-mean stages ----
            def seg_stage(sname, t_idx, t_lid, t_w, T, tiles_per_blk, src_full,
                          dst_sh, final):
                gb_cur = None
                kb_cur = 0
                tglob = 0
                for blk, nt in enumerate(tiles_per_blk):
                    ps = ppool.tile([P, DW], f32, tag="acc", name=f"acc_{sname}_b{blk}")
                    for ti in range(nt):
                        t = tglob + ti
                        if t % KB == 0:
                            kb_cur = min(KB, T - t)
                            gb_cur = gpool.tile([P, KB * DW], bf16, tag="gb",
                                                name=f"gb_{sname}_{t}")
                            nc.gpsimd.indirect_dma_start(
                                out=gb_cur[:, 0:kb_cur * DW], out_offset=None,
                                in_=src_full[:],
                                in_offset=bass.IndirectOffsetOnAxis(
                                    ap=t_idx[:, t:t + kb_cur], axis=0))
                        slot = t % KB
                        sel = selpool.tile([P, P], bf16, tag="sel",
                                           name=f"sel_{sname}_{t}")
                        nc.vector.scalar_tensor_tensor(
                            out=sel[:], in0=t_iota[:], scalar=t_lid[:, t:t + 1],
                            in1=t_w[:, t:t + 1].to_broadcast([P, P]),
                            op0=mybir.AluOpType.is_equal, op1=mybir.AluOpType.mult)
                        nc.tensor.matmul(out=ps[:, 0:DW], lhsT=sel[:],
                                         rhs=gb_cur[:, slot * DW:(slot + 1) * DW],
                                         start=(ti == 0), stop=(ti == nt - 1))
                    tglob += nt
                    # finalize block: mean = num / max(den, 1e-12)
                    den = wpool.tile([P, 1], f32, tag="den")
                    nc.vector.tensor_scalar(out=den[:], in0=ps[:, D:DW],
                                            scalar1=1e-12, scalar2=None,
                                            op0=mybir.AluOpType.max)
                    rec = wpool.tile([P, 1], f32, tag="rec")
                    nc.vector.reciprocal(out=rec[:], in_=den[:])
                    if not final:
                        ob = opool.tile([P, DW], bf16, tag="yo")
                        nc.scalar.mul(ob[:, 0:D], ps[:, 0:D], rec[:, 0:1])
                        nc.vector.memset(ob[:, D:DW], 1.0)
                        nc.sync.dma_start(out=dst_sh[blk * P:(blk + 1) * P, :], in_=ob[:])
                    else:
                        mean = wpool.tile([P, D], f32, tag="mean")
                        nc.scalar.mul(mean[:], ps[:, 0:D], rec[:, 0:1])
                        mx = wpool.tile([P, 1], f32, tag="mx")
                        nc.vector.tensor_reduce(out=mx[:], in_=mean[:],
                                                axis=mybir.AxisListType.X,
                                                op=mybir.AluOpType.max)
                        nmx = wpool.tile([P, 1], f32, tag="nmx")
                        nc.vector.tensor_scalar(out=nmx[:], in0=mx[:], scalar1=-1.0,
                                                scalar2=None, op0=mybir.AluOpType.mult)
                        ex = wpool.tile([P, D], f32, tag="ex")
                        ssum = wpool.tile([P, 1], f32, tag="ssum")
                        nc.scalar.activation(out=ex[:], in_=mean[:],
                                             func=mybir.ActivationFunctionType.Exp,
                                             bias=nmx[:, 0:1], accum_out=ssum[:])
                        rs = wpool.tile([P, 1], f32, tag="rs")
                        nc.vector.reciprocal(out=rs[:], in_=ssum[:])
                        fo = opool.tile([P, D], bf16, tag="fo")
                        nc.scalar.mul(fo[:], ex[:], rs[:, 0:1])
                        nc.sync.dma_start(out=p_out[blk * P:(blk + 1) * P, :], in_=fo[:])

            seg_stage("s1", t_idxA, t_lidA, t_wA, TA, tilesA, x0_full, y1_sh, False)
            nc.gpsimd.collective_compute("AllGather", mybir.AluOpType.bypass,
                                         replica_groups=rg, ins=[y1_sh[:]], outs=[y1_full[:]])
            seg_stage("s2", t_idxB, t_lidB, t_wB, TB, tilesB, y1_full, x1_sh, False)
            nc.gpsimd.collective_compute("AllGather", mybir.AluOpType.bypass,
                                         replica_groups=rg, ins=[x1_sh[:]], outs=[x1_full[:]])
            seg_stage("s3", t_idxA, t_lidA, t_wA, TA, tilesA, x1_full, y2_sh, False)
            nc.gpsimd.collective_compute("AllGather", mybir.AluOpType.bypass,
                                         replica_groups=rg, ins=[y2_sh[:]], outs=[y2_full[:]])
            seg_stage("s4", t_idxB, t_lidB, t_wB, TB, tilesB, y2_full, None, True)

    nc.finalize()

    in_maps = []
    for k in range(NC):
        m = {"featsT": featsT[k], "W": Wb, "b_mat": b_mat, "iota": iota,
             "idxA": stA[0][k], "lidA": stA[1][k], "wA": stA[2][k],
             "idxB": stB[0][k], "lidB": stB[1][k], "wB": stB[2][k]}
        in_maps.append(m)

    import time as _time
    res = run_bass_kernel_spmd(nc, in_maps, list(range(NC)), trace=False)
    exec_ns = None
    if trace:
        times = []
        for _ in range(3):
            t0 = _time.time()
            res = run_bass_kernel_spmd(nc, in_maps, list(range(NC)), trace=False)
            times.append(_time.time() - t0)
        exec_ns = int(min(times) * 1e9)
    out = np.concatenate(
        [np.asarray(res.results[k]["out"][:V_SH], np.float32) for k in range(NC)],
        axis=0)
    return out, exec_ns


def kernel(**inputs):
    out, _ = _build_and_run(inputs, trace=False)
    return out


# revision 13
# speedup vs baseline: 1.6054x; 1.6054x over previous
"""Hypergraph 2-hop message passing (gnn_message_passing) on 8 trn2 cores.

Pipeline: x0 = feats@W+b -> y1 = v2e-mean(x0) -> x1 = e2v-mean(y1)
          -> y2 = v2e-mean(x1) -> x2 = e2v-mean(y2) -> softmax(x2)

Sharding: vertices and edges row-sharded across 8 cores. Each segment-mean
stage partitions incidence pairs by destination shard; sources are fetched
with batched indirect DMA (row gather, 32 tiles per SWDGE op) from an
AllGather'd full table held in Shared HBM. Segment sums are computed with
one-hot selection matmuls accumulating in PSUM; a ones-column appended to
every table row yields the denominator in the same matmul. All tables and
matmul operands are bf16 (f32 PSUM accumulation).
"""
import math
import os
import numpy as np
import ml_dtypes

# Persistent XLA compilation cache: repeat calls (and repeat processes) skip
# recompiling the unchanged program. Must be set before jax initializes.
os.environ.setdefault("JAX_COMPILATION_CACHE_DIR", "/tmp/jax_cache_kernel")

BF16 = ml_dtypes.bfloat16
_SHARED_AG = os.environ.get("K_SHARED", "1") == "1"

N = 200_000
E = 50_000
NNZ = 2_000_000
F_IN = 256
D = 128
DW = D + 1                 # feature row + ones column (denominator)
NC = 8
P = 128
KB = int(os.environ.get("K_KB", "1"))  # tiles per indirect gather (HW rejects >1)

V_SH = N // NC             # 25000
E_SH = E // NC             # 6250
V_BLK = math.ceil(V_SH / P)    # 196
E_BLK = math.ceil(E_SH / P)    # 49
V_PAD = V_BLK * P          # 25088
E_PAD = E_BLK * P          # 6272


def _build_stage(dst, src_rows, w, n_dst_sh, n_blk):
    """Partition pairs by destination shard, sort by destination, pad each
    128-destination block to a common (max-over-cores) tile count.

    dst: global destination ids [NNZ]; src_rows: padded-table row ids [NNZ]
    Returns per-core [128, T] arrays (idx int32, lid bf16, w bf16), T, and
    per-block tile counts (shared across cores).
    """
    core_of = dst // n_dst_sh
    loc = dst % n_dst_sh
    per_core = []
    counts = np.zeros((NC, n_blk), np.int64)
    for k in range(NC):
        m = core_of == k
        lo = loc[m]
        order = np.argsort(lo, kind="stable")
        lo = lo[order]
        sr = src_rows[m][order]
        wk = w[m][order]
        blk = lo // P
        counts[k] = np.bincount(blk, minlength=n_blk)
        per_core.append((lo, sr, wk))
    tiles = np.maximum(np.ceil(counts / P).astype(np.int64).max(axis=0), 1)  # [n_blk]
    T = int(tiles.sum())
    starts = np.zeros(n_blk + 1, np.int64)
    starts[1:] = np.cumsum(tiles * P)
    idx_all, lid_all, w_all = [], [], []
    for k in range(NC):
        lo, sr, wk = per_core[k]
        idx = np.zeros(T * P, np.int32)
        lid = np.zeros(T * P, np.float32)
        ww = np.zeros(T * P, np.float32)
        bstart = np.zeros(n_blk + 1, np.int64)
        bstart[1:] = np.cumsum(counts[k])
        for b in range(n_blk):
            s, e = bstart[b], bstart[b + 1]
            o = starts[b]
            idx[o:o + (e - s)] = sr[s:e]
            lid[o:o + (e - s)] = (lo[s:e] - b * P).astype(np.float32)
            ww[o:o + (e - s)] = wk[s:e]
        idx_all.append(np.ascontiguousarray(idx.reshape(T, P).T))
        lid_all.append(np.ascontiguousarray(lid.reshape(T, P).T.astype(BF16)))
        w_all.append(np.ascontiguousarray(ww.reshape(T, P).T.astype(BF16)))
    return idx_all, lid_all, w_all, T, [int(t) for t in tiles]


def _pad_rows_v(v):
    return (v // V_SH) * V_PAD + (v % V_SH)


def _pad_rows_e(e):
    return (e // E_SH) * E_PAD + (e % E_SH)


def _build_and_run(inputs, trace=False):
    import jax
    try:
        jax.config.update("jax_compilation_cache_dir", "/tmp/jax_cache_kernel")
    except Exception:
        pass
    from concourse import bacc, bass, mybir, tile
    from concourse.bass_utils import run_bass_kernel_spmd

    feats = np.asarray(inputs["feats"], np.float32)
    W = np.asarray(inputs["W"], np.float32)
    b = np.asarray(inputs["b"], np.float32)
    pair_v = np.asarray(inputs["pair_v"], np.int32)
    pair_e = np.asarray(inputs["pair_e"], np.int32)
    v2e_w = np.asarray(inputs["v2e_weight"], np.float32)
    e2v_w = np.asarray(inputs["e2v_weight"], np.float32)

    # ---------------- host-side index prep ----------------
    src_x = _pad_rows_v(pair_v)
    src_y = _pad_rows_e(pair_e)
    # stage A: v2e (edge destinations), used for hops 1 and 2
    stA = _build_stage(pair_e.astype(np.int64), src_x, v2e_w, E_SH, E_BLK)
    # stage B: e2v (vertex destinations), used for hops 1 and 2
    stB = _build_stage(pair_v.astype(np.int64), src_y, e2v_w, V_SH, V_BLK)
    TA, tilesA = stA[3], stA[4]
    TB, tilesB = stB[3], stB[4]

    # featsT per core, padded, bf16
    featsT = []
    for k in range(NC):
        sh = np.zeros((V_PAD, F_IN), np.float32)
        sh[:V_SH] = feats[k * V_SH:(k + 1) * V_SH]
        featsT.append(np.ascontiguousarray(sh.T.astype(BF16)))
    Wb = np.ascontiguousarray(W.astype(BF16))
    b_mat = np.broadcast_to(b[None, :], (P, D)).copy().astype(np.float32)
    iota = np.broadcast_to(np.arange(P, dtype=np.float32)[None, :], (P, P)).astype(BF16).copy()

    # ---------------- build program ----------------
    f32 = mybir.dt.float32
    bf16 = mybir.dt.bfloat16
    i32 = mybir.dt.int32
    nc = bacc.Bacc("TRN2", target_bir_lowering=False, debug=False, num_devices=NC)
    p_ftT = nc.declare_dram_parameter("featsT", [F_IN, V_PAD], bf16, isOutput=False)
    p_W = nc.declare_dram_parameter("W", [F_IN, D], bf16, isOutput=False)
    p_b = nc.declare_dram_parameter("b_mat", [P, D], f32, isOutput=False)
    p_iota = nc.declare_dram_parameter("iota", [P, P], bf16, isOutput=False)
    p_idxA = nc.declare_dram_parameter("idxA", [P, TA], i32, isOutput=False)
    p_lidA = nc.declare_dram_parameter("lidA", [P, TA], bf16, isOutput=False)
    p_wA = nc.declare_dram_parameter("wA", [P, TA], bf16, isOutput=False)
    p_idxB = nc.declare_dram_parameter("idxB", [P, TB], i32, isOutput=False)
    p_lidB = nc.declare_dram_parameter("lidB", [P, TB], bf16, isOutput=False)
    p_wB = nc.declare_dram_parameter("wB", [P, TB], bf16, isOutput=False)
    p_out = nc.declare_dram_parameter("out", [V_PAD, D], bf16, isOutput=True)

    x0_sh = nc.dram_tensor("x0_sh", [V_PAD, DW], bf16)
    x0_full = nc.dram_tensor("x0_full", [NC * V_PAD, DW], bf16, addr_space="Shared" if _SHARED_AG else "Local")
    y1_sh = nc.dram_tensor("y1_sh", [E_PAD, DW], bf16)
    y1_full = nc.dram_tensor("y1_full", [NC * E_PAD, DW], bf16, addr_space="Shared" if _SHARED_AG else "Local")
    x1_sh = nc.dram_tensor("x1_sh", [V_PAD, DW], bf16)
    x1_full = nc.dram_tensor("x1_full", [NC * V_PAD, DW], bf16, addr_space="Shared" if _SHARED_AG else "Local")
    y2_sh = nc.dram_tensor("y2_sh", [E_PAD, DW], bf16)
    y2_full = nc.dram_tensor("y2_full", [NC * E_PAD, DW], bf16, addr_space="Shared" if _SHARED_AG else "Local")

    rg = [list(range(NC))]
    with tile.TileContext(nc) as tc:
        with tc.tile_pool(name="const", bufs=1) as cpool, \
             tc.tile_pool(name="tabs", bufs=1) as tpool, \
             tc.tile_pool(name="fstream", bufs=4) as fpool, \
             tc.tile_pool(name="gath", bufs=4) as gpool, \
             tc.tile_pool(name="sel", bufs=8) as selpool, \
             tc.tile_pool(name="fin", bufs=4) as wpool, \
             tc.tile_pool(name="outp", bufs=4) as opool, \
             tc.tile_pool(name="psum", bufs=6, space="PSUM") as ppool:

            t_W = cpool.tile([P, 2, D], bf16, tag="wt")
            nc.sync.dma_start(out=t_W[:, 0, :], in_=p_W[0:128, :])
            nc.sync.dma_start(out=t_W[:, 1, :], in_=p_W[128:256, :])
            t_b = cpool.tile([P, D], f32, tag="bmat")
            nc.sync.dma_start(out=t_b[:], in_=p_b[:])
            t_iota = cpool.tile([P, P], bf16, tag="iota")
            nc.sync.dma_start(out=t_iota[:], in_=p_iota[:])

            t_idxA = tpool.tile([P, TA], i32, tag="idxA")
            t_lidA = tpool.tile([P, TA], bf16, tag="lidA")
            t_wA = tpool.tile([P, TA], bf16, tag="wA")
            nc.sync.dma_start(out=t_idxA[:], in_=p_idxA[:])
            nc.sync.dma_start(out=t_lidA[:], in_=p_lidA[:])
            nc.sync.dma_start(out=t_wA[:], in_=p_wA[:])
            t_idxB = tpool.tile([P, TB], i32, tag="idxB")
            t_lidB = tpool.tile([P, TB], bf16, tag="lidB")
            t_wB = tpool.tile([P, TB], bf16, tag="wB")
            nc.sync.dma_start(out=t_idxB[:], in_=p_idxB[:])
            nc.sync.dma_start(out=t_lidB[:], in_=p_lidB[:])
            nc.sync.dma_start(out=t_wB[:], in_=p_wB[:])

            # ---- stage 0: x0 = feats @ W + b (featsT pre-transposed) ----
            for rt in range(V_BLK):
                ft = fpool.tile([P, 2, P], bf16, tag="ft")
                nc.sync.dma_start(out=ft[:, 0, :], in_=p_ftT[0:128, rt * P:(rt + 1) * P])
                nc.sync.dma_start(out=ft[:, 1, :], in_=p_ftT[128:256, rt * P:(rt + 1) * P])
                ps = ppool.tile([P, DW], f32, tag="acc", name=f"ps0_{rt}")
                nc.tensor.matmul(out=ps[:, 0:D], lhsT=ft[:, 0, :], rhs=t_W[:, 0, :], start=True, stop=False)
                nc.tensor.matmul(out=ps[:, 0:D], lhsT=ft[:, 1, :], rhs=t_W[:, 1, :], start=False, stop=True)
                ob = opool.tile([P, DW], bf16, tag="x0o")
                nc.vector.tensor_tensor(out=ob[:, 0:D], in0=ps[:, 0:D], in1=t_b[:], op=mybir.AluOpType.add)
                nc.vector.memset(ob[:, D:DW], 1.0)
                nc.sync.dma_start(out=x0_sh[rt * P:(rt + 1) * P, :], in_=ob[:])
            nc.gpsimd.collective_compute("AllGather", mybir.AluOpType.bypass,
                                         replica_groups=rg, ins=[x0_sh[:]], outs=[x0_full[:]])

            # ---- segment-mean stages ----
            def seg_stage(sname, t_idx, t_lid, t_w, T, tiles_per_blk, src_full,
                          dst_sh, final):
                gb_cur = None
                kb_cur = 0
                tglob = 0
                for blk, nt in enumerate(tiles_per_blk):
                    ps = ppool.tile([P, DW], f32, tag="acc", name=f"acc_{sname}_b{blk}")
                    for ti in range(nt):
                        t = tglob + ti
                        if t % KB == 0:
                            kb_cur = min(KB, T - t)
                            gb_cur = gpool.tile([P, KB * DW], bf16, tag="gb",
                                                name=f"gb_{sname}_{t}")
                            nc.gpsimd.indirect_dma_start(
                                out=gb_cur[:, 0:kb_cur * DW], out_offset=None,
                                in_=src_full[:],
                                in_offset=bass.IndirectOffsetOnAxis(
                                    ap=t_idx[:, t:t + kb_cur], axis=0))
                        slot = t % KB
                        sel = selpool.tile([P, P], bf16, tag="sel",
                                           name=f"sel_{sname}_{t}")
                        nc.vector.scalar_tensor_tensor(
                            out=sel[:], in0=t_iota[:], scalar=t_lid[:, t:t + 1],
                            in1=t_w[:, t:t + 1].to_broadcast([P, P]),
                            op0=mybir.AluOpType.is_equal, op1=mybir.AluOpType.mult)
                        nc.tensor.matmul(out=ps[:, 0:DW], lhsT=sel[:],
                                         rhs=gb_cur[:, slot * DW:(slot + 1) * DW],
                                         start=(ti == 0), stop=(ti == nt - 1))
                    tglob += nt
                    # finalize block: mean = num / max(den, 1e-12)
                    den = wpool.tile([P, 1], f32, tag="den")
                    nc.vector.tensor_scalar(out=den[:], in0=ps[:, D:DW],
                                            scalar1=1e-12, scalar2=None,
                                            op0=mybir.AluOpType.max)
                    rec = wpool.tile([P, 1], f32, tag="rec")
                    nc.vector.reciprocal(out=rec[:], in_=den[:])
                    if not final:
                        ob = opool.tile([P, DW], bf16, tag="yo")
                        nc.scalar.mul(ob[:, 0:D], ps[:, 0:D], rec[:, 0:1])
                        nc.vector.memset(ob[:, D:DW], 1.0)
                        nc.sync.dma_start(out=dst_sh[blk * P:(blk + 1) * P, :], in_=ob[:])
                    else:
                        mean = wpool.tile([P, D], f32, tag="mean")
                        nc.scalar.mul(mean[:], ps[:, 0:D], rec[:, 0:1])
                        mx = wpool.tile([P, 1], f32, tag="mx")
                        nc.vector.tensor_reduce(out=mx[:], in_=mean[:],
                                                axis=mybir.AxisListType.X,
                                                op=mybir.AluOpType.max)
                        nmx = wpool.tile([P, 1], f32, tag="nmx")
                        nc.vector.tensor_scalar(out=nmx[:], in0=mx[:], scalar1=-1.0,
                                                scalar2=None, op0=mybir.AluOpType.mult)
                        ex = wpool.tile([P, D], f32, tag="ex")
                        ssum = wpool.tile([P, 1], f32, tag="ssum")
                        nc.scalar.activation(out=ex[:], in_=mean[:],
                                             func=mybir.ActivationFunctionType.Exp,
                                             bias=nmx[:, 0:1], accum_out=ssum[:])
                        rs = wpool.tile([P, 1], f32, tag="rs")
                        nc.vector.reciprocal(out=rs[:], in_=ssum[:])
                        fo = opool.tile([P, D], bf16, tag="fo")
                        nc.scalar.mul(fo[:], ex[:], rs[:, 0:1])
                        nc.sync.dma_start(out=p_out[blk * P:(blk + 1) * P, :], in_=fo[:])

            seg_stage("s1", t_idxA, t_lidA, t_wA, TA, tilesA, x0_full, y1_sh, False)
            nc.gpsimd.collective_compute("AllGather", mybir.AluOpType.bypass,
                                         replica_groups=rg, ins=[y1_sh[:]], outs=[y1_full[:]])
            seg_stage("s2", t_idxB, t_lidB, t_wB, TB, tilesB, y1_full, x1_sh, False)
            nc.gpsimd.collective_compute("AllGather", mybir.AluOpType.bypass,
                                         replica_groups=rg, ins=[x1_sh[:]], outs=[x1_full[:]])
            seg_stage("s3", t_idxA, t_lidA, t_wA, TA, tilesA, x1_full, y2_sh, False)
            nc.gpsimd.collective_compute("AllGather", mybir.AluOpType.bypass,
                                         replica_groups=rg, ins=[y2_sh[:]], outs=[y2_full[:]])
            seg_stage("s4", t_idxB, t_lidB, t_wB, TB, tilesB, y2_full, None, True)

    nc.finalize()

    in_maps = []
    for k in range(NC):
        m = {"featsT": featsT[k], "W": Wb, "b_mat": b_mat, "iota": iota,
             "idxA": stA[0][k], "lidA": stA[1][k], "wA": stA[2][k],
             "idxB": stB[0][k], "lidB": stB[1][k], "wB": stB[2][k]}
        in_maps.append(m)

    import time as _time
    res = run_bass_kernel_spmd(nc, in_maps, list(range(NC)), trace=False)
    exec_ns = None
    if trace:
        times = []
        for _ in range(3):
            t0 = _time.time()
            res = run_bass_kernel_spmd(nc, in_maps, list(range(NC)), trace=False)
            times.append(_time.time() - t0)
        exec_ns = int(min(times) * 1e9)
    out = np.concatenate(
        [np.asarray(res.results[k]["out"][:V_SH], np.float32) for k in range(NC)],
        axis=0)
    return out, exec_ns


def kernel(**inputs):
    out, _ = _build_and_run(inputs, trace=False)
    return out


# revision 19
# speedup vs baseline: 2.6811x; 1.6701x over previous
"""Hypergraph 2-hop message passing (gnn_message_passing) on 8 trn2 cores.

Pipeline: x0 = feats@W+b -> y1 = v2e-mean(x0) -> x1 = e2v-mean(y1)
          -> y2 = v2e-mean(x1) -> x2 = e2v-mean(y2) -> softmax(x2)

Sharding: vertices and edges row-sharded across 8 cores. Each segment-mean
stage partitions incidence pairs by destination shard; sources are fetched
with batched indirect DMA (row gather, 32 tiles per SWDGE op) from an
AllGather'd full table held in Shared HBM. Segment sums are computed with
one-hot selection matmuls accumulating in PSUM; a ones-column appended to
every table row yields the denominator in the same matmul. All tables and
matmul operands are bf16 (f32 PSUM accumulation).
"""
import math
import os
import numpy as np
import ml_dtypes

# Persistent XLA compilation cache: repeat calls (and repeat processes) skip
# recompiling the unchanged program. Must be set before jax initializes.
os.environ.setdefault("JAX_COMPILATION_CACHE_DIR", "/tmp/jax_cache_kernel")

BF16 = ml_dtypes.bfloat16
FP8 = ml_dtypes.float8_e4m3
_SHARED_AG = os.environ.get("K_SHARED", "1") == "1"

N = 200_000
E = 50_000
NNZ = 2_000_000
F_IN = 256
D = 128
DW = D + 1                 # feature row + ones column (denominator)
NC = 8
P = 128
KB = int(os.environ.get("K_KB", "1"))  # tiles per indirect gather (HW rejects >1)

V_SH = N // NC             # 25000
E_SH = E // NC             # 6250
V_BLK = math.ceil(V_SH / P)    # 196
E_BLK = math.ceil(E_SH / P)    # 49
V_PAD = V_BLK * P          # 25088
E_PAD = E_BLK * P          # 6272


def _build_stage(dst, src_rows, w, n_dst_sh, n_blk):
    """Partition pairs by destination shard, sort by destination, pad each
    128-destination block to a common (max-over-cores) tile count.

    dst: global destination ids [NNZ]; src_rows: padded-table row ids [NNZ]
    Returns per-core [128, T] arrays (idx int32, lid bf16, w bf16), T, and
    per-block tile counts (shared across cores).
    """
    core_of = dst // n_dst_sh
    loc = dst % n_dst_sh
    per_core = []
    counts = np.zeros((NC, n_blk), np.int64)
    for k in range(NC):
        m = core_of == k
        lo = loc[m]
        order = np.argsort(lo, kind="stable")
        lo = lo[order]
        sr = src_rows[m][order]
        wk = w[m][order]
        blk = lo // P
        counts[k] = np.bincount(blk, minlength=n_blk)
        per_core.append((lo, sr, wk))
    tiles = np.maximum(np.ceil(counts / P).astype(np.int64).max(axis=0), 1)  # [n_blk]
    T = int(tiles.sum())
    starts = np.zeros(n_blk + 1, np.int64)
    starts[1:] = np.cumsum(tiles * P)
    pk_all, w_all = [], []
    for k in range(NC):
        lo, sr, wk = per_core[k]
        idx = np.zeros(T * P, np.int32)
        lid = np.zeros(T * P, np.int32)
        ww = np.zeros(T * P, np.float32)
        bstart = np.zeros(n_blk + 1, np.int64)
        bstart[1:] = np.cumsum(counts[k])
        for b in range(n_blk):
            s, e = bstart[b], bstart[b + 1]
            o = starts[b]
            idx[o:o + (e - s)] = sr[s:e]
            lid[o:o + (e - s)] = lo[s:e] - b * P
            ww[o:o + (e - s)] = wk[s:e]
        # pack: low 18 bits = gather row, bits 18..24 = local dst id
        packed = idx | (lid << 18)
        pk_all.append(np.ascontiguousarray(packed.reshape(T, P).T))
        w_all.append(np.ascontiguousarray(ww.reshape(T, P).T.astype(FP8)))
    return pk_all, w_all, T, [int(t) for t in tiles]


def _pad_rows_v(v):
    return (v // V_SH) * V_PAD + (v % V_SH)


def _pad_rows_e(e):
    return (e // E_SH) * E_PAD + (e % E_SH)


def _build_and_run(inputs, trace=False):
    import jax
    try:
        jax.config.update("jax_compilation_cache_dir", "/tmp/jax_cache_kernel")
    except Exception:
        pass
    from concourse import bacc, bass, mybir, tile
    from concourse.bass_utils import run_bass_kernel_spmd

    feats = np.asarray(inputs["feats"], np.float32)
    W = np.asarray(inputs["W"], np.float32)
    b = np.asarray(inputs["b"], np.float32)
    pair_v = np.asarray(inputs["pair_v"], np.int32)
    pair_e = np.asarray(inputs["pair_e"], np.int32)
    v2e_w = np.asarray(inputs["v2e_weight"], np.float32)
    e2v_w = np.asarray(inputs["e2v_weight"], np.float32)

    # ---------------- host-side index prep ----------------
    src_x = _pad_rows_v(pair_v)
    src_y = _pad_rows_e(pair_e)
    # stage A: v2e (edge destinations), used for hops 1 and 2
    stA = _build_stage(pair_e.astype(np.int64), src_x, v2e_w, E_SH, E_BLK)
    # stage B: e2v (vertex destinations), used for hops 1 and 2
    stB = _build_stage(pair_v.astype(np.int64), src_y, e2v_w, V_SH, V_BLK)
    TA, tilesA = stA[2], stA[3]
    TB, tilesB = stB[2], stB[3]

    # Consolidated per-core params (one buffer per dtype — each host->device
    # transfer costs ~70ms fixed over the axon tunnel, so fewer is faster).
    # fp8_all [P, 2*V_PAD + TA + TB]: featsT half0 | half1 | wA | wB
    # i32_all [P, TA + TB]:           packed idx+lid A | B
    # bf16_all [P, 4*P]:              W half0 | W half1 | iota | b
    CF8_FT1 = V_PAD
    CF8_WA = 2 * V_PAD
    CF8_WB = CF8_WA + TA
    fp8_all = []
    for k in range(NC):
        sh = np.zeros((V_PAD, F_IN), np.float32)
        sh[:V_SH] = feats[k * V_SH:(k + 1) * V_SH]
        ftT = sh.T.astype(FP8)  # [F_IN, V_PAD]
        buf = np.empty((P, 2 * V_PAD + TA + TB), FP8)
        buf[:, :V_PAD] = ftT[:P]
        buf[:, CF8_FT1:CF8_WA] = ftT[P:]
        buf[:, CF8_WA:CF8_WB] = stA[1][k]
        buf[:, CF8_WB:] = stB[1][k]
        fp8_all.append(buf)
    i32_all = [np.concatenate([stA[0][k], stB[0][k]], axis=1) for k in range(NC)]
    Wb = W.astype(BF16)
    iota = np.broadcast_to(np.arange(P, dtype=np.float32)[None, :], (P, P)).astype(BF16)
    b_mat = np.broadcast_to(b[None, :], (P, D)).astype(BF16)
    bf16_all = np.ascontiguousarray(
        np.concatenate([Wb[:P], Wb[P:], iota, b_mat], axis=1))

    # ---------------- build program ----------------
    f32 = mybir.dt.float32
    bf16 = mybir.dt.bfloat16
    i32 = mybir.dt.int32
    fp8 = mybir.dt.float8e4
    nc = bacc.Bacc("TRN2", target_bir_lowering=False, debug=False, num_devices=NC)
    p_f8 = nc.declare_dram_parameter("fp8_all", [P, 2 * V_PAD + TA + TB], fp8, isOutput=False)
    p_i32 = nc.declare_dram_parameter("i32_all", [P, TA + TB], i32, isOutput=False)
    p_b16 = nc.declare_dram_parameter("bf16_all", [P, 4 * P], bf16, isOutput=False)
    p_out = nc.declare_dram_parameter("out", [V_PAD, D], bf16, isOutput=True)

    x0_sh = nc.dram_tensor("x0_sh", [V_PAD, DW], bf16)
    x0_full = nc.dram_tensor("x0_full", [NC * V_PAD, DW], bf16, addr_space="Shared" if _SHARED_AG else "Local")
    y1_sh = nc.dram_tensor("y1_sh", [E_PAD, DW], bf16)
    y1_full = nc.dram_tensor("y1_full", [NC * E_PAD, DW], bf16, addr_space="Shared" if _SHARED_AG else "Local")
    x1_sh = nc.dram_tensor("x1_sh", [V_PAD, DW], bf16)
    x1_full = nc.dram_tensor("x1_full", [NC * V_PAD, DW], bf16, addr_space="Shared" if _SHARED_AG else "Local")
    y2_sh = nc.dram_tensor("y2_sh", [E_PAD, DW], bf16)
    y2_full = nc.dram_tensor("y2_full", [NC * E_PAD, DW], bf16, addr_space="Shared" if _SHARED_AG else "Local")

    rg = [list(range(NC))]
    with tile.TileContext(nc) as tc:
        with tc.tile_pool(name="const", bufs=1) as cpool, \
             tc.tile_pool(name="tabs", bufs=1) as tpool, \
             tc.tile_pool(name="fstream", bufs=4) as fpool, \
             tc.tile_pool(name="gath", bufs=4) as gpool, \
             tc.tile_pool(name="sel", bufs=8) as selpool, \
             tc.tile_pool(name="fin", bufs=4) as wpool, \
             tc.tile_pool(name="outp", bufs=4) as opool, \
             tc.tile_pool(name="psum", bufs=6, space="PSUM") as ppool:

            t_b16 = cpool.tile([P, 4 * P], bf16, tag="b16")
            nc.sync.dma_start(out=t_b16[:], in_=p_b16[:])
            t_W0 = t_b16[:, 0:D]
            t_W1 = t_b16[:, D:2 * D]
            t_iota = t_b16[:, 2 * D:2 * D + P]
            t_b = t_b16[:, 2 * D + P:2 * D + 2 * P]

            # unpack stage tables: fp8 weights -> bf16, packed idx+lid -> idx/lid
            t_w8 = tpool.tile([P, TA + TB], fp8, tag="w8")
            nc.sync.dma_start(out=t_w8[:], in_=p_f8[:, CF8_WA:])
            t_w = tpool.tile([P, TA + TB], bf16, tag="w")
            nc.vector.tensor_copy(out=t_w[:], in_=t_w8[:])
            t_pk = tpool.tile([P, TA + TB], i32, tag="pk")
            nc.sync.dma_start(out=t_pk[:], in_=p_i32[:])
            t_idx = tpool.tile([P, TA + TB], i32, tag="idx")
            nc.vector.tensor_scalar(out=t_idx[:], in0=t_pk[:], scalar1=0x3FFFF,
                                    scalar2=None, op0=mybir.AluOpType.bitwise_and)
            t_hi = tpool.tile([P, TA + TB], i32, tag="hi")
            nc.vector.tensor_scalar(out=t_hi[:], in0=t_pk[:], scalar1=18,
                                    scalar2=None,
                                    op0=mybir.AluOpType.logical_shift_right)
            t_lid = tpool.tile([P, TA + TB], bf16, tag="lid")
            nc.vector.tensor_copy(out=t_lid[:], in_=t_hi[:])
            t_idxA, t_idxB = t_idx[:, 0:TA], t_idx[:, TA:]
            t_lidA, t_lidB = t_lid[:, 0:TA], t_lid[:, TA:]
            t_wA, t_wB = t_w[:, 0:TA], t_w[:, TA:]

            # ---- stage 0: x0 = feats @ W + b (featsT pre-transposed, fp8) ----
            for rt in range(V_BLK):
                ft8 = fpool.tile([P, 2, P], fp8, tag="ft8")
                nc.sync.dma_start(out=ft8[:, 0, :], in_=p_f8[:, rt * P:(rt + 1) * P])
                nc.sync.dma_start(out=ft8[:, 1, :],
                                  in_=p_f8[:, CF8_FT1 + rt * P:CF8_FT1 + (rt + 1) * P])
                ft = fpool.tile([P, 2, P], bf16, tag="ft")
                nc.vector.tensor_copy(out=ft[:], in_=ft8[:])
                ps = ppool.tile([P, DW], f32, tag="acc", name=f"ps0_{rt}")
                nc.tensor.matmul(out=ps[:, 0:D], lhsT=ft[:, 0, :], rhs=t_W0, start=True, stop=False)
                nc.tensor.matmul(out=ps[:, 0:D], lhsT=ft[:, 1, :], rhs=t_W1, start=False, stop=True)
                ob = opool.tile([P, DW], bf16, tag="x0o")
                nc.vector.tensor_tensor(out=ob[:, 0:D], in0=ps[:, 0:D], in1=t_b, op=mybir.AluOpType.add)
                nc.vector.memset(ob[:, D:DW], 1.0)
                nc.sync.dma_start(out=x0_sh[rt * P:(rt + 1) * P, :], in_=ob[:])
            nc.gpsimd.collective_compute("AllGather", mybir.AluOpType.bypass,
                                         replica_groups=rg, ins=[x0_sh[:]], outs=[x0_full[:]])

            # ---- segment-mean stages ----
            def seg_stage(sname, t_idx, t_lid, t_w, T, tiles_per_blk, src_full,
                          dst_sh, final):
                gb_cur = None
                kb_cur = 0
                tglob = 0
                for blk, nt in enumerate(tiles_per_blk):
                    ps = ppool.tile([P, DW], f32, tag="acc", name=f"acc_{sname}_b{blk}")
                    for ti in range(nt):
                        t = tglob + ti
                        if t % KB == 0:
                            kb_cur = min(KB, T - t)
                            gb_cur = gpool.tile([P, KB * DW], bf16, tag="gb",
                                                name=f"gb_{sname}_{t}")
                            nc.gpsimd.indirect_dma_start(
                                out=gb_cur[:, 0:kb_cur * DW], out_offset=None,
                                in_=src_full[:],
                                in_offset=bass.IndirectOffsetOnAxis(
                                    ap=t_idx[:, t:t + kb_cur], axis=0))
                        slot = t % KB
                        sel = selpool.tile([P, P], bf16, tag="sel",
                                           name=f"sel_{sname}_{t}")
                        nc.vector.scalar_tensor_tensor(
                            out=sel[:], in0=t_iota[:], scalar=t_lid[:, t:t + 1],
                            in1=t_w[:, t:t + 1].to_broadcast([P, P]),
                            op0=mybir.AluOpType.is_equal, op1=mybir.AluOpType.mult)
                        nc.tensor.matmul(out=ps[:, 0:DW], lhsT=sel[:],
                                         rhs=gb_cur[:, slot * DW:(slot + 1) * DW],
                                         start=(ti == 0), stop=(ti == nt - 1))
                    tglob += nt
                    # finalize block: mean = num / max(den, 1e-12)
                    den = wpool.tile([P, 1], f32, tag="den")
                    nc.vector.tensor_scalar(out=den[:], in0=ps[:, D:DW],
                                            scalar1=1e-12, scalar2=None,
                                            op0=mybir.AluOpType.max)
                    rec = wpool.tile([P, 1], f32, tag="rec")
                    nc.vector.reciprocal(out=rec[:], in_=den[:])
                    if not final:
                        ob = opool.tile([P, DW], bf16, tag="yo")
                        nc.scalar.mul(ob[:, 0:D], ps[:, 0:D], rec[:, 0:1])
                        nc.vector.memset(ob[:, D:DW], 1.0)
                        nc.sync.dma_start(out=dst_sh[blk * P:(blk + 1) * P, :], in_=ob[:])
                    else:
                        mean = wpool.tile([P, D], f32, tag="mean")
                        nc.scalar.mul(mean[:], ps[:, 0:D], rec[:, 0:1])
                        mx = wpool.tile([P, 1], f32, tag="mx")
                        nc.vector.tensor_reduce(out=mx[:], in_=mean[:],
                                                axis=mybir.AxisListType.X,
                                                op=mybir.AluOpType.max)
                        nmx = wpool.tile([P, 1], f32, tag="nmx")
                        nc.vector.tensor_scalar(out=nmx[:], in0=mx[:], scalar1=-1.0,
                                                scalar2=None, op0=mybir.AluOpType.mult)
                        ex = wpool.tile([P, D], f32, tag="ex")
                        ssum = wpool.tile([P, 1], f32, tag="ssum")
                        nc.scalar.activation(out=ex[:], in_=mean[:],
                                             func=mybir.ActivationFunctionType.Exp,
                                             bias=nmx[:, 0:1], accum_out=ssum[:])
                        rs = wpool.tile([P, 1], f32, tag="rs")
                        nc.vector.reciprocal(out=rs[:], in_=ssum[:])
                        fo = opool.tile([P, D], bf16, tag="fo")
                        nc.scalar.mul(fo[:], ex[:], rs[:, 0:1])
                        nc.sync.dma_start(out=p_out[blk * P:(blk + 1) * P, :], in_=fo[:])

            seg_stage("s1", t_idxA, t_lidA, t_wA, TA, tilesA, x0_full, y1_sh, False)
            nc.gpsimd.collective_compute("AllGather", mybir.AluOpType.bypass,
                                         replica_groups=rg, ins=[y1_sh[:]], outs=[y1_full[:]])
            seg_stage("s2", t_idxB, t_lidB, t_wB, TB, tilesB, y1_full, x1_sh, False)
            nc.gpsimd.collective_compute("AllGather", mybir.AluOpType.bypass,
                                         replica_groups=rg, ins=[x1_sh[:]], outs=[x1_full[:]])
            seg_stage("s3", t_idxA, t_lidA, t_wA, TA, tilesA, x1_full, y2_sh, False)
            nc.gpsimd.collective_compute("AllGather", mybir.AluOpType.bypass,
                                         replica_groups=rg, ins=[y2_sh[:]], outs=[y2_full[:]])
            seg_stage("s4", t_idxB, t_lidB, t_wB, TB, tilesB, y2_full, None, True)

    nc.finalize()

    in_maps = [{"fp8_all": fp8_all[k], "i32_all": i32_all[k],
                "bf16_all": bf16_all} for k in range(NC)]

    import time as _time
    res = run_bass_kernel_spmd(nc, in_maps, list(range(NC)), trace=False)
    exec_ns = None
    if trace:
        times = []
        for _ in range(3):
            t0 = _time.time()
            res = run_bass_kernel_spmd(nc, in_maps, list(range(NC)), trace=False)
            times.append(_time.time() - t0)
        exec_ns = int(min(times) * 1e9)
    out = np.concatenate(
        [np.asarray(res.results[k]["out"][:V_SH], np.float32) for k in range(NC)],
        axis=0)
    return out, exec_ns


def kernel(**inputs):
    out, _ = _build_and_run(inputs, trace=False)
    return out


# revision 22
# speedup vs baseline: 2.9101x; 1.0854x over previous
"""Hypergraph 2-hop message passing (gnn_message_passing) on 8 trn2 cores.

Pipeline: x0 = feats@W+b -> y1 = v2e-mean(x0) -> x1 = e2v-mean(y1)
          -> y2 = v2e-mean(x1) -> x2 = e2v-mean(y2) -> softmax(x2)

Sharding: vertices and edges row-sharded across 8 cores. Each segment-mean
stage partitions incidence pairs by destination shard; sources are fetched
with batched indirect DMA (row gather, 32 tiles per SWDGE op) from an
AllGather'd full table held in Shared HBM. Segment sums are computed with
one-hot selection matmuls accumulating in PSUM; a ones-column appended to
every table row yields the denominator in the same matmul. All tables and
matmul operands are bf16 (f32 PSUM accumulation).
"""
import math
import os
import numpy as np
import ml_dtypes

# Persistent XLA compilation cache: repeat calls (and repeat processes) skip
# recompiling the unchanged program. Must be set before jax initializes.
os.environ.setdefault("JAX_COMPILATION_CACHE_DIR", "/tmp/jax_cache_kernel")

BF16 = ml_dtypes.bfloat16
FP8 = ml_dtypes.float8_e4m3
_SHARED_AG = os.environ.get("K_SHARED", "1") == "1"

N = 200_000
E = 50_000
NNZ = 2_000_000
F_IN = 256
D = 128
DW = D + 1                 # feature row + ones column (denominator)
NC = 8
P = 128
KB = int(os.environ.get("K_KB", "1"))  # tiles per indirect gather (HW rejects >1)
KT = 8                     # tiles per batched sel-matrix build

V_SH = N // NC             # 25000
E_SH = E // NC             # 6250
V_BLK = math.ceil(V_SH / P)    # 196
E_BLK = math.ceil(E_SH / P)    # 49
V_PAD = V_BLK * P          # 25088
E_PAD = E_BLK * P          # 6272


def _build_stage(dst, src_rows, w, n_dst_sh, n_blk):
    """Partition pairs by destination shard, sort by destination, pad each
    128-destination block to a common (max-over-cores) tile count.

    dst: global destination ids [NNZ]; src_rows: padded-table row ids [NNZ]
    Returns per-core [128, T] arrays (idx int32, lid bf16, w bf16), T, and
    per-block tile counts (shared across cores).
    """
    core_of = dst // n_dst_sh
    loc = dst % n_dst_sh
    per_core = []
    counts = np.zeros((NC, n_blk), np.int64)
    for k in range(NC):
        m = core_of == k
        lo = loc[m]
        order = np.argsort(lo, kind="stable")
        lo = lo[order]
        sr = src_rows[m][order]
        wk = w[m][order]
        blk = lo // P
        counts[k] = np.bincount(blk, minlength=n_blk)
        per_core.append((lo, sr, wk))
    tiles = np.maximum(np.ceil(counts / P).astype(np.int64).max(axis=0), 1)  # [n_blk]
    T = int(tiles.sum())
    starts = np.zeros(n_blk + 1, np.int64)
    starts[1:] = np.cumsum(tiles * P)
    pk_all, w_all = [], []
    for k in range(NC):
        lo, sr, wk = per_core[k]
        idx = np.zeros(T * P, np.int32)
        lid = np.zeros(T * P, np.int32)
        ww = np.zeros(T * P, np.float32)
        bstart = np.zeros(n_blk + 1, np.int64)
        bstart[1:] = np.cumsum(counts[k])
        for b in range(n_blk):
            s, e = bstart[b], bstart[b + 1]
            o = starts[b]
            idx[o:o + (e - s)] = sr[s:e]
            lid[o:o + (e - s)] = lo[s:e] - b * P
            ww[o:o + (e - s)] = wk[s:e]
        # pack: low 18 bits = gather row, bits 18..24 = local dst id
        packed = idx | (lid << 18)
        pk_all.append(np.ascontiguousarray(packed.reshape(T, P).T))
        w_all.append(np.ascontiguousarray(ww.reshape(T, P).T.astype(FP8)))
    return pk_all, w_all, T, [int(t) for t in tiles]


def _pad_rows_v(v):
    return (v // V_SH) * V_PAD + (v % V_SH)


def _pad_rows_e(e):
    return (e // E_SH) * E_PAD + (e % E_SH)


def _build_and_run(inputs, trace=False):
    import jax
    try:
        jax.config.update("jax_compilation_cache_dir", "/tmp/jax_cache_kernel")
    except Exception:
        pass
    from concourse import bacc, bass, mybir, tile
    from concourse.bass_utils import run_bass_kernel_spmd

    feats = np.asarray(inputs["feats"], np.float32)
    W = np.asarray(inputs["W"], np.float32)
    b = np.asarray(inputs["b"], np.float32)
    pair_v = np.asarray(inputs["pair_v"], np.int32)
    pair_e = np.asarray(inputs["pair_e"], np.int32)
    v2e_w = np.asarray(inputs["v2e_weight"], np.float32)
    e2v_w = np.asarray(inputs["e2v_weight"], np.float32)

    # ---------------- host-side index prep ----------------
    src_x = _pad_rows_v(pair_v)
    src_y = _pad_rows_e(pair_e)
    # stage A: v2e (edge destinations), used for hops 1 and 2
    stA = _build_stage(pair_e.astype(np.int64), src_x, v2e_w, E_SH, E_BLK)
    # stage B: e2v (vertex destinations), used for hops 1 and 2
    stB = _build_stage(pair_v.astype(np.int64), src_y, e2v_w, V_SH, V_BLK)
    TA, tilesA = stA[2], stA[3]
    TB, tilesB = stB[2], stB[3]

    # Consolidated per-core params (one buffer per dtype — each host->device
    # transfer costs ~70ms fixed over the axon tunnel, so fewer is faster).
    # fp8_all [P, 2*V_PAD + TA + TB]: featsT half0 | half1 | wA | wB
    # i32_all [P, TA + TB]:           packed idx+lid A | B
    # bf16_all [P, 4*P]:              W half0 | W half1 | iota | b
    CF8_FT1 = V_PAD
    CF8_WA = 2 * V_PAD
    CF8_WB = CF8_WA + TA
    fp8_all = []
    for k in range(NC):
        sh = np.zeros((V_PAD, F_IN), np.float32)
        sh[:V_SH] = feats[k * V_SH:(k + 1) * V_SH]
        ftT = sh.T.astype(FP8)  # [F_IN, V_PAD]
        buf = np.empty((P, 2 * V_PAD + TA + TB), FP8)
        buf[:, :V_PAD] = ftT[:P]
        buf[:, CF8_FT1:CF8_WA] = ftT[P:]
        buf[:, CF8_WA:CF8_WB] = stA[1][k]
        buf[:, CF8_WB:] = stB[1][k]
        fp8_all.append(buf)
    i32_all = [np.concatenate([stA[0][k], stB[0][k]], axis=1) for k in range(NC)]
    Wb = W.astype(BF16)
    iota = np.broadcast_to(np.arange(P, dtype=np.float32)[None, :], (P, P)).astype(BF16)
    b_mat = np.broadcast_to(b[None, :], (P, D)).astype(BF16)
    bf16_all = np.ascontiguousarray(
        np.concatenate([Wb[:P], Wb[P:], iota, b_mat], axis=1))

    # ---------------- build program ----------------
    f32 = mybir.dt.float32
    bf16 = mybir.dt.bfloat16
    i32 = mybir.dt.int32
    fp8 = mybir.dt.float8e4
    nc = bacc.Bacc("TRN2", target_bir_lowering=False, debug=False, num_devices=NC)
    p_f8 = nc.declare_dram_parameter("fp8_all", [P, 2 * V_PAD + TA + TB], fp8, isOutput=False)
    p_i32 = nc.declare_dram_parameter("i32_all", [P, TA + TB], i32, isOutput=False)
    p_b16 = nc.declare_dram_parameter("bf16_all", [P, 4 * P], bf16, isOutput=False)
    p_out = nc.declare_dram_parameter("out", [V_PAD, D], bf16, isOutput=True)

    x0_sh = nc.dram_tensor("x0_sh", [V_PAD, DW], bf16)
    x0_full = nc.dram_tensor("x0_full", [NC * V_PAD, DW], bf16, addr_space="Shared" if _SHARED_AG else "Local")
    y1_sh = nc.dram_tensor("y1_sh", [E_PAD, DW], bf16)
    y1_full = nc.dram_tensor("y1_full", [NC * E_PAD, DW], bf16, addr_space="Shared" if _SHARED_AG else "Local")
    x1_sh = nc.dram_tensor("x1_sh", [V_PAD, DW], bf16)
    x1_full = nc.dram_tensor("x1_full", [NC * V_PAD, DW], bf16, addr_space="Shared" if _SHARED_AG else "Local")
    y2_sh = nc.dram_tensor("y2_sh", [E_PAD, DW], bf16)
    y2_full = nc.dram_tensor("y2_full", [NC * E_PAD, DW], bf16, addr_space="Shared" if _SHARED_AG else "Local")

    rg = [list(range(NC))]
    with tile.TileContext(nc) as tc:
        with tc.tile_pool(name="const", bufs=1) as cpool, \
             tc.tile_pool(name="tabs", bufs=1) as tpool, \
             tc.tile_pool(name="fstream", bufs=4) as fpool, \
             tc.tile_pool(name="gath", bufs=4) as gpool, \
             tc.tile_pool(name="sel", bufs=8) as selpool, \
             tc.tile_pool(name="fin", bufs=4) as wpool, \
             tc.tile_pool(name="outp", bufs=4) as opool, \
             tc.tile_pool(name="psum", bufs=6, space="PSUM") as ppool:

            t_b16 = cpool.tile([P, 4 * P], bf16, tag="b16")
            nc.sync.dma_start(out=t_b16[:], in_=p_b16[:])
            t_W0 = t_b16[:, 0:D]
            t_W1 = t_b16[:, D:2 * D]
            t_iota = t_b16[:, 2 * D:2 * D + P]
            t_b = t_b16[:, 2 * D + P:2 * D + 2 * P]

            # unpack stage tables: fp8 weights -> bf16, packed idx+lid -> idx/lid
            t_w8 = tpool.tile([P, TA + TB], fp8, tag="w8")
            nc.sync.dma_start(out=t_w8[:], in_=p_f8[:, CF8_WA:])
            t_w = tpool.tile([P, TA + TB], bf16, tag="w")
            nc.vector.tensor_copy(out=t_w[:], in_=t_w8[:])
            t_pk = tpool.tile([P, TA + TB], i32, tag="pk")
            nc.sync.dma_start(out=t_pk[:], in_=p_i32[:])
            t_idx = tpool.tile([P, TA + TB], i32, tag="idx")
            nc.vector.tensor_scalar(out=t_idx[:], in0=t_pk[:], scalar1=0x3FFFF,
                                    scalar2=None, op0=mybir.AluOpType.bitwise_and)
            t_hi = tpool.tile([P, TA + TB], i32, tag="hi")
            nc.vector.tensor_scalar(out=t_hi[:], in0=t_pk[:], scalar1=18,
                                    scalar2=None,
                                    op0=mybir.AluOpType.logical_shift_right)
            t_lid = tpool.tile([P, TA + TB], bf16, tag="lid")
            nc.vector.tensor_copy(out=t_lid[:], in_=t_hi[:])
            t_idxA, t_idxB = t_idx[:, 0:TA], t_idx[:, TA:]
            t_lidA, t_lidB = t_lid[:, 0:TA], t_lid[:, TA:]
            t_wA, t_wB = t_w[:, 0:TA], t_w[:, TA:]

            # iota replicated KT times for batched sel builds
            t_iota2 = cpool.tile([P, KT, P], bf16, tag="iota2")
            for j in range(KT):
                nc.sync.dma_start(out=t_iota2[:, j, :], in_=p_b16[:, 2 * D:2 * D + P])

            # ---- stage 0: x0 = feats @ W + b (featsT pre-transposed, fp8) ----
            for rt in range(V_BLK):
                ft8 = fpool.tile([P, 2, P], fp8, tag="ft8")
                nc.sync.dma_start(out=ft8[:, 0, :], in_=p_f8[:, rt * P:(rt + 1) * P])
                nc.sync.dma_start(out=ft8[:, 1, :],
                                  in_=p_f8[:, CF8_FT1 + rt * P:CF8_FT1 + (rt + 1) * P])
                ft = fpool.tile([P, 2, P], bf16, tag="ft")
                nc.vector.tensor_copy(out=ft[:], in_=ft8[:])
                ps = ppool.tile([P, DW], f32, tag="acc", name=f"ps0_{rt}")
                nc.tensor.matmul(out=ps[:, 0:D], lhsT=ft[:, 0, :], rhs=t_W0, start=True, stop=False)
                nc.tensor.matmul(out=ps[:, 0:D], lhsT=ft[:, 1, :], rhs=t_W1, start=False, stop=True)
                ob = opool.tile([P, DW], bf16, tag="x0o")
                nc.vector.tensor_tensor(out=ob[:, 0:D], in0=ps[:, 0:D], in1=t_b, op=mybir.AluOpType.add)
                nc.vector.memset(ob[:, D:DW], 1.0)
                nc.sync.dma_start(out=x0_sh[rt * P:(rt + 1) * P, :], in_=ob[:])
            nc.gpsimd.collective_compute("AllGather", mybir.AluOpType.bypass,
                                         replica_groups=rg, ins=[x0_sh[:]], outs=[x0_full[:]])

            # ---- segment-mean stages ----
            def seg_stage(sname, t_idx, t_lid, t_w, T, tiles_per_blk, src_full,
                          dst_sh, final):
                gb_cur = None
                selg_cur = None
                kb_cur = 0
                tglob = 0
                for blk, nt in enumerate(tiles_per_blk):
                    ps = ppool.tile([P, DW], f32, tag="acc", name=f"acc_{sname}_b{blk}")
                    for ti in range(nt):
                        t = tglob + ti
                        if t % KB == 0:
                            kb_cur = min(KB, T - t)
                            gb_cur = gpool.tile([P, KB * DW], bf16, tag="gb",
                                                name=f"gb_{sname}_{t}")
                            nc.gpsimd.indirect_dma_start(
                                out=gb_cur[:, 0:kb_cur * DW], out_offset=None,
                                in_=src_full[:],
                                in_offset=bass.IndirectOffsetOnAxis(
                                    ap=t_idx[:, t:t + kb_cur], axis=0))
                        slot = t % KB
                        if t % KT == 0:
                            kt = min(KT, T - t)
                            selg_cur = selpool.tile([P, KT, P], bf16, tag="selg",
                                                    name=f"selg_{sname}_{t}")
                            nc.vector.tensor_tensor(
                                out=selg_cur[:, 0:kt, :], in0=t_iota2[:, 0:kt, :],
                                in1=t_lid[:, t:t + kt].to_broadcast([P, kt, P]),
                                op=mybir.AluOpType.is_equal)
                            nc.vector.tensor_tensor(
                                out=selg_cur[:, 0:kt, :], in0=selg_cur[:, 0:kt, :],
                                in1=t_w[:, t:t + kt].to_broadcast([P, kt, P]),
                                op=mybir.AluOpType.mult)
                        nc.tensor.matmul(out=ps[:, 0:DW], lhsT=selg_cur[:, t % KT, :],
                                         rhs=gb_cur[:, slot * DW:(slot + 1) * DW],
                                         start=(ti == 0), stop=(ti == nt - 1))
                    tglob += nt
                    # finalize block: mean = num / max(den, 1e-12)
                    den = wpool.tile([P, 1], f32, tag="den")
                    nc.vector.tensor_scalar(out=den[:], in0=ps[:, D:DW],
                                            scalar1=1e-12, scalar2=None,
                                            op0=mybir.AluOpType.max)
                    rec = wpool.tile([P, 1], f32, tag="rec")
                    nc.vector.reciprocal(out=rec[:], in_=den[:])
                    if not final:
                        ob = opool.tile([P, DW], bf16, tag="yo")
                        nc.scalar.mul(ob[:, 0:D], ps[:, 0:D], rec[:, 0:1])
                        nc.vector.memset(ob[:, D:DW], 1.0)
                        nc.sync.dma_start(out=dst_sh[blk * P:(blk + 1) * P, :], in_=ob[:])
                    else:
                        mean = wpool.tile([P, D], f32, tag="mean")
                        nc.scalar.mul(mean[:], ps[:, 0:D], rec[:, 0:1])
                        mx = wpool.tile([P, 1], f32, tag="mx")
                        nc.vector.tensor_reduce(out=mx[:], in_=mean[:],
                                                axis=mybir.AxisListType.X,
                                                op=mybir.AluOpType.max)
                        nmx = wpool.tile([P, 1], f32, tag="nmx")
                        nc.vector.tensor_scalar(out=nmx[:], in0=mx[:], scalar1=-1.0,
                                                scalar2=None, op0=mybir.AluOpType.mult)
                        ex = wpool.tile([P, D], f32, tag="ex")
                        ssum = wpool.tile([P, 1], f32, tag="ssum")
                        nc.scalar.activation(out=ex[:], in_=mean[:],
                                             func=mybir.ActivationFunctionType.Exp,
                                             bias=nmx[:, 0:1], accum_out=ssum[:])
                        rs = wpool.tile([P, 1], f32, tag="rs")
                        nc.vector.reciprocal(out=rs[:], in_=ssum[:])
                        fo = opool.tile([P, D], bf16, tag="fo")
                        nc.scalar.mul(fo[:], ex[:], rs[:, 0:1])
                        nc.sync.dma_start(out=p_out[blk * P:(blk + 1) * P, :], in_=fo[:])

            seg_stage("s1", t_idxA, t_lidA, t_wA, TA, tilesA, x0_full, y1_sh, False)
            nc.gpsimd.collective_compute("AllGather", mybir.AluOpType.bypass,
                                         replica_groups=rg, ins=[y1_sh[:]], outs=[y1_full[:]])
            seg_stage("s2", t_idxB, t_lidB, t_wB, TB, tilesB, y1_full, x1_sh, False)
            nc.gpsimd.collective_compute("AllGather", mybir.AluOpType.bypass,
                                         replica_groups=rg, ins=[x1_sh[:]], outs=[x1_full[:]])
            seg_stage("s3", t_idxA, t_lidA, t_wA, TA, tilesA, x1_full, y2_sh, False)
            nc.gpsimd.collective_compute("AllGather", mybir.AluOpType.bypass,
                                         replica_groups=rg, ins=[y2_sh[:]], outs=[y2_full[:]])
            seg_stage("s4", t_idxB, t_lidB, t_wB, TB, tilesB, y2_full, None, True)

    nc.finalize()

    in_maps = [{"fp8_all": fp8_all[k], "i32_all": i32_all[k],
                "bf16_all": bf16_all} for k in range(NC)]

    import time as _time
    res = run_bass_kernel_spmd(nc, in_maps, list(range(NC)), trace=False)
    exec_ns = None
    if trace:
        times = []
        for _ in range(3):
            t0 = _time.time()
            res = run_bass_kernel_spmd(nc, in_maps, list(range(NC)), trace=False)
            times.append(_time.time() - t0)
        exec_ns = int(min(times) * 1e9)
    out = np.concatenate(
        [np.asarray(res.results[k]["out"][:V_SH], np.float32) for k in range(NC)],
        axis=0)
    return out, exec_ns


def kernel(**inputs):
    out, _ = _build_and_run(inputs, trace=False)
    return out


# revision 25
# speedup vs baseline: 3.8074x; 1.3083x over previous
"""Hypergraph 2-hop message passing (gnn_message_passing) on 8 trn2 cores.

Pipeline: x0 = feats@W+b -> y1 = v2e-mean(x0) -> x1 = e2v-mean(y1)
          -> y2 = v2e-mean(x1) -> x2 = e2v-mean(y2) -> softmax(x2)

Sharding: vertices and edges row-sharded across 8 cores. Each segment-mean
stage partitions incidence pairs by destination shard; sources are fetched
with batched indirect DMA (row gather, 32 tiles per SWDGE op) from an
AllGather'd full table held in Shared HBM. Segment sums are computed with
one-hot selection matmuls accumulating in PSUM; a ones-column appended to
every table row yields the denominator in the same matmul. All tables and
matmul operands are bf16 (f32 PSUM accumulation).
"""
import math
import os
import numpy as np
import ml_dtypes

# Persistent XLA compilation cache: repeat calls (and repeat processes) skip
# recompiling the unchanged program. Must be set before jax initializes.
os.environ.setdefault("JAX_COMPILATION_CACHE_DIR", "/tmp/jax_cache_kernel")

BF16 = ml_dtypes.bfloat16
FP8 = ml_dtypes.float8_e4m3
_SHARED_AG = os.environ.get("K_SHARED", "1") == "1"

N = 200_000
E = 50_000
NNZ = 2_000_000
F_IN = 256
D = 128
DW = D + 1                 # feature row + ones column (denominator)
NC = 8
P = 128
KB = int(os.environ.get("K_KB", "1"))  # tiles per indirect gather (HW rejects >1)
KT = 8                     # tiles per batched sel-matrix build

V_SH = N // NC             # 25000
E_SH = E // NC             # 6250
V_BLK = math.ceil(V_SH / P)    # 196
E_BLK = math.ceil(E_SH / P)    # 49
V_PAD = V_BLK * P          # 25088
E_PAD = E_BLK * P          # 6272


def _build_stage(dst, src_rows, w, n_dst_sh, n_blk):
    """Partition pairs by destination shard, sort by destination, pad each
    128-destination block to a common (max-over-cores) tile count.

    dst: global destination ids [NNZ]; src_rows: padded-table row ids [NNZ]
    Returns per-core [128, T] arrays (idx int32, lid bf16, w bf16), T, and
    per-block tile counts (shared across cores).
    """
    core_of = dst // n_dst_sh
    loc = dst % n_dst_sh
    per_core = []
    counts = np.zeros((NC, n_blk), np.int64)
    for k in range(NC):
        m = core_of == k
        lo = loc[m]
        order = np.argsort(lo, kind="stable")
        lo = lo[order]
        sr = src_rows[m][order]
        wk = w[m][order]
        blk = lo // P
        counts[k] = np.bincount(blk, minlength=n_blk)
        per_core.append((lo, sr, wk))
    tiles = np.maximum(np.ceil(counts / P).astype(np.int64).max(axis=0), 1)  # [n_blk]
    T = int(tiles.sum())
    starts = np.zeros(n_blk + 1, np.int64)
    starts[1:] = np.cumsum(tiles * P)
    pk_all, w_all = [], []
    for k in range(NC):
        lo, sr, wk = per_core[k]
        idx = np.zeros(T * P, np.int32)
        lid = np.zeros(T * P, np.int32)
        ww = np.zeros(T * P, np.float32)
        bstart = np.zeros(n_blk + 1, np.int64)
        bstart[1:] = np.cumsum(counts[k])
        for b in range(n_blk):
            s, e = bstart[b], bstart[b + 1]
            o = starts[b]
            idx[o:o + (e - s)] = sr[s:e]
            lid[o:o + (e - s)] = lo[s:e] - b * P
            ww[o:o + (e - s)] = wk[s:e]
        # pack: low 18 bits = gather row, bits 18..24 = local dst id
        packed = idx | (lid << 18)
        pk_all.append(np.ascontiguousarray(packed.reshape(T, P).T))
        w_all.append(np.ascontiguousarray(ww.reshape(T, P).T.astype(FP8)))
    return pk_all, w_all, T, [int(t) for t in tiles]


def _pad_rows_v(v):
    return (v // V_SH) * V_PAD + (v % V_SH)


def _pad_rows_e(e):
    return (e // E_SH) * E_PAD + (e % E_SH)


def _build_and_run(inputs, trace=False):
    import jax
    try:
        jax.config.update("jax_compilation_cache_dir", "/tmp/jax_cache_kernel")
    except Exception:
        pass
    from concourse import bacc, bass, mybir, tile
    from concourse.bass_utils import run_bass_kernel_spmd

    feats = np.asarray(inputs["feats"], np.float32)
    W = np.asarray(inputs["W"], np.float32)
    b = np.asarray(inputs["b"], np.float32)
    pair_v = np.asarray(inputs["pair_v"], np.int32)
    pair_e = np.asarray(inputs["pair_e"], np.int32)
    v2e_w = np.asarray(inputs["v2e_weight"], np.float32)
    e2v_w = np.asarray(inputs["e2v_weight"], np.float32)

    # ---------------- host-side index prep ----------------
    src_x = _pad_rows_v(pair_v)
    src_y = _pad_rows_e(pair_e)
    # stage A: v2e (edge destinations), used for hops 1 and 2
    stA = _build_stage(pair_e.astype(np.int64), src_x, v2e_w, E_SH, E_BLK)
    # stage B: e2v (vertex destinations), used for hops 1 and 2
    stB = _build_stage(pair_v.astype(np.int64), src_y, e2v_w, V_SH, V_BLK)
    TA, tilesA = stA[2], stA[3]
    TB, tilesB = stB[2], stB[3]

    # Consolidated per-core params (one buffer per dtype — each host->device
    # transfer costs ~70ms fixed over the axon tunnel, so fewer is faster).
    # fp8_all [P, 2*V_PAD + TA + TB]: featsT half0 | half1 | wA | wB
    # i32_all [P, TA + TB]:           packed idx+lid A | B
    # bf16_all [P, 4*P]:              W half0 | W half1 | iota | b
    CF8_FT1 = V_PAD
    CF8_WA = 2 * V_PAD
    CF8_WB = CF8_WA + TA
    fp8_all = []
    for k in range(NC):
        sh = np.zeros((V_PAD, F_IN), np.float32)
        sh[:V_SH] = feats[k * V_SH:(k + 1) * V_SH]
        ftT = sh.T.astype(FP8)  # [F_IN, V_PAD]
        buf = np.empty((P, 2 * V_PAD + TA + TB), FP8)
        buf[:, :V_PAD] = ftT[:P]
        buf[:, CF8_FT1:CF8_WA] = ftT[P:]
        buf[:, CF8_WA:CF8_WB] = stA[1][k]
        buf[:, CF8_WB:] = stB[1][k]
        fp8_all.append(buf)
    i32_all = [np.concatenate([stA[0][k], stB[0][k]], axis=1) for k in range(NC)]
    Wb = W.astype(BF16)
    iota = np.broadcast_to(np.arange(P, dtype=np.float32)[None, :], (P, P)).astype(BF16)
    b_mat = np.broadcast_to(b[None, :], (P, D)).astype(BF16)
    bf16_all = np.ascontiguousarray(
        np.concatenate([Wb[:P], Wb[P:], iota, b_mat], axis=1))

    # ---------------- build program ----------------
    f32 = mybir.dt.float32
    bf16 = mybir.dt.bfloat16
    i32 = mybir.dt.int32
    fp8 = mybir.dt.float8e4
    nc = bacc.Bacc("TRN2", target_bir_lowering=False, debug=False, num_devices=NC)
    p_f8 = nc.declare_dram_parameter("fp8_all", [P, 2 * V_PAD + TA + TB], fp8, isOutput=False)
    p_i32 = nc.declare_dram_parameter("i32_all", [P, TA + TB], i32, isOutput=False)
    p_b16 = nc.declare_dram_parameter("bf16_all", [P, 4 * P], bf16, isOutput=False)
    # output = pre-softmax logits scaled x16, fp8 (host normalizes; the scale
    # keeps small logits in e4m3's normal range)
    p_out = nc.declare_dram_parameter("out", [V_PAD, D], fp8, isOutput=True)

    x0_sh = nc.dram_tensor("x0_sh", [V_PAD, DW], bf16)
    x0_full = nc.dram_tensor("x0_full", [NC * V_PAD, DW], bf16, addr_space="Shared" if _SHARED_AG else "Local")
    y1_sh = nc.dram_tensor("y1_sh", [E_PAD, DW], bf16)
    y1_full = nc.dram_tensor("y1_full", [NC * E_PAD, DW], bf16, addr_space="Shared" if _SHARED_AG else "Local")
    x1_sh = nc.dram_tensor("x1_sh", [V_PAD, DW], bf16)
    x1_full = nc.dram_tensor("x1_full", [NC * V_PAD, DW], bf16, addr_space="Shared" if _SHARED_AG else "Local")
    y2_sh = nc.dram_tensor("y2_sh", [E_PAD, DW], bf16)
    y2_full = nc.dram_tensor("y2_full", [NC * E_PAD, DW], bf16, addr_space="Shared" if _SHARED_AG else "Local")

    rg = [list(range(NC))]
    with tile.TileContext(nc) as tc:
        with tc.tile_pool(name="const", bufs=1) as cpool, \
             tc.tile_pool(name="tabs", bufs=1) as tpool, \
             tc.tile_pool(name="fstream", bufs=4) as fpool, \
             tc.tile_pool(name="gath", bufs=4) as gpool, \
             tc.tile_pool(name="sel", bufs=8) as selpool, \
             tc.tile_pool(name="fin", bufs=4) as wpool, \
             tc.tile_pool(name="outp", bufs=4) as opool, \
             tc.tile_pool(name="psum", bufs=6, space="PSUM") as ppool:

            t_b16 = cpool.tile([P, 4 * P], bf16, tag="b16")
            nc.sync.dma_start(out=t_b16[:], in_=p_b16[:])
            t_W0 = t_b16[:, 0:D]
            t_W1 = t_b16[:, D:2 * D]
            t_iota = t_b16[:, 2 * D:2 * D + P]
            t_b = t_b16[:, 2 * D + P:2 * D + 2 * P]

            # unpack stage tables: fp8 weights -> bf16, packed idx+lid -> idx/lid
            t_w8 = tpool.tile([P, TA + TB], fp8, tag="w8")
            nc.sync.dma_start(out=t_w8[:], in_=p_f8[:, CF8_WA:])
            t_w = tpool.tile([P, TA + TB], bf16, tag="w")
            nc.vector.tensor_copy(out=t_w[:], in_=t_w8[:])
            t_pk = tpool.tile([P, TA + TB], i32, tag="pk")
            nc.sync.dma_start(out=t_pk[:], in_=p_i32[:])
            t_idx = tpool.tile([P, TA + TB], i32, tag="idx")
            nc.vector.tensor_scalar(out=t_idx[:], in0=t_pk[:], scalar1=0x3FFFF,
                                    scalar2=None, op0=mybir.AluOpType.bitwise_and)
            t_hi = tpool.tile([P, TA + TB], i32, tag="hi")
            nc.vector.tensor_scalar(out=t_hi[:], in0=t_pk[:], scalar1=18,
                                    scalar2=None,
                                    op0=mybir.AluOpType.logical_shift_right)
            t_lid = tpool.tile([P, TA + TB], bf16, tag="lid")
            nc.vector.tensor_copy(out=t_lid[:], in_=t_hi[:])
            t_idxA, t_idxB = t_idx[:, 0:TA], t_idx[:, TA:]
            t_lidA, t_lidB = t_lid[:, 0:TA], t_lid[:, TA:]
            t_wA, t_wB = t_w[:, 0:TA], t_w[:, TA:]

            # iota replicated KT times for batched sel builds
            t_iota2 = cpool.tile([P, KT, P], bf16, tag="iota2")
            for j in range(KT):
                nc.sync.dma_start(out=t_iota2[:, j, :], in_=p_b16[:, 2 * D:2 * D + P])

            # ---- stage 0: x0 = feats @ W + b (featsT pre-transposed, fp8) ----
            for rt in range(V_BLK):
                ft8 = fpool.tile([P, 2, P], fp8, tag="ft8")
                nc.sync.dma_start(out=ft8[:, 0, :], in_=p_f8[:, rt * P:(rt + 1) * P])
                nc.sync.dma_start(out=ft8[:, 1, :],
                                  in_=p_f8[:, CF8_FT1 + rt * P:CF8_FT1 + (rt + 1) * P])
                ft = fpool.tile([P, 2, P], bf16, tag="ft")
                nc.vector.tensor_copy(out=ft[:], in_=ft8[:])
                ps = ppool.tile([P, DW], f32, tag="acc", name=f"ps0_{rt}")
                nc.tensor.matmul(out=ps[:, 0:D], lhsT=ft[:, 0, :], rhs=t_W0, start=True, stop=False)
                nc.tensor.matmul(out=ps[:, 0:D], lhsT=ft[:, 1, :], rhs=t_W1, start=False, stop=True)
                ob = opool.tile([P, DW], bf16, tag="x0o")
                nc.vector.tensor_tensor(out=ob[:, 0:D], in0=ps[:, 0:D], in1=t_b, op=mybir.AluOpType.add)
                nc.vector.memset(ob[:, D:DW], 1.0)
                nc.sync.dma_start(out=x0_sh[rt * P:(rt + 1) * P, :], in_=ob[:])
            nc.gpsimd.collective_compute("AllGather", mybir.AluOpType.bypass,
                                         replica_groups=rg, ins=[x0_sh[:]], outs=[x0_full[:]])

            # ---- segment-mean stages ----
            def seg_stage(sname, t_idx, t_lid, t_w, T, tiles_per_blk, src_full,
                          dst_sh, final):
                gb_cur = None
                selg_cur = None
                kb_cur = 0
                tglob = 0
                for blk, nt in enumerate(tiles_per_blk):
                    ps = ppool.tile([P, DW], f32, tag="acc", name=f"acc_{sname}_b{blk}")
                    for ti in range(nt):
                        t = tglob + ti
                        if t % KB == 0:
                            kb_cur = min(KB, T - t)
                            gb_cur = gpool.tile([P, KB * DW], bf16, tag="gb",
                                                name=f"gb_{sname}_{t}")
                            nc.gpsimd.indirect_dma_start(
                                out=gb_cur[:, 0:kb_cur * DW], out_offset=None,
                                in_=src_full[:],
                                in_offset=bass.IndirectOffsetOnAxis(
                                    ap=t_idx[:, t:t + kb_cur], axis=0))
                        slot = t % KB
                        if t % KT == 0:
                            kt = min(KT, T - t)
                            selg_cur = selpool.tile([P, KT, P], bf16, tag="selg",
                                                    name=f"selg_{sname}_{t}")
                            nc.vector.tensor_tensor(
                                out=selg_cur[:, 0:kt, :], in0=t_iota2[:, 0:kt, :],
                                in1=t_lid[:, t:t + kt].to_broadcast([P, kt, P]),
                                op=mybir.AluOpType.is_equal)
                            nc.vector.tensor_tensor(
                                out=selg_cur[:, 0:kt, :], in0=selg_cur[:, 0:kt, :],
                                in1=t_w[:, t:t + kt].to_broadcast([P, kt, P]),
                                op=mybir.AluOpType.mult)
                        nc.tensor.matmul(out=ps[:, 0:DW], lhsT=selg_cur[:, t % KT, :],
                                         rhs=gb_cur[:, slot * DW:(slot + 1) * DW],
                                         start=(ti == 0), stop=(ti == nt - 1))
                    tglob += nt
                    # finalize block: mean = num / max(den, 1e-12)
                    den = wpool.tile([P, 1], f32, tag="den")
                    if not final:
                        nc.vector.tensor_scalar(out=den[:], in0=ps[:, D:DW],
                                                scalar1=1e-12, scalar2=None,
                                                op0=mybir.AluOpType.max)
                        rec = wpool.tile([P, 1], f32, tag="rec")
                        nc.vector.reciprocal(out=rec[:], in_=den[:])
                        ob = opool.tile([P, DW], bf16, tag="yo")
                        nc.scalar.mul(ob[:, 0:D], ps[:, 0:D], rec[:, 0:1])
                        nc.vector.memset(ob[:, D:DW], 1.0)
                        nc.sync.dma_start(out=dst_sh[blk * P:(blk + 1) * P, :], in_=ob[:])
                    else:
                        # den/16 so the reciprocal bakes in the x16 logit scale
                        nc.vector.tensor_scalar(out=den[:], in0=ps[:, D:DW],
                                                scalar1=1e-12, scalar2=1.0 / 16.0,
                                                op0=mybir.AluOpType.max,
                                                op1=mybir.AluOpType.mult)
                        rec = wpool.tile([P, 1], f32, tag="rec")
                        nc.vector.reciprocal(out=rec[:], in_=den[:])
                        fo = opool.tile([P, D], fp8, tag="fo")
                        nc.scalar.mul(fo[:], ps[:, 0:D], rec[:, 0:1])
                        nc.sync.dma_start(out=p_out[blk * P:(blk + 1) * P, :], in_=fo[:])

            seg_stage("s1", t_idxA, t_lidA, t_wA, TA, tilesA, x0_full, y1_sh, False)
            nc.gpsimd.collective_compute("AllGather", mybir.AluOpType.bypass,
                                         replica_groups=rg, ins=[y1_sh[:]], outs=[y1_full[:]])
            seg_stage("s2", t_idxB, t_lidB, t_wB, TB, tilesB, y1_full, x1_sh, False)
            nc.gpsimd.collective_compute("AllGather", mybir.AluOpType.bypass,
                                         replica_groups=rg, ins=[x1_sh[:]], outs=[x1_full[:]])
            seg_stage("s3", t_idxA, t_lidA, t_wA, TA, tilesA, x1_full, y2_sh, False)
            nc.gpsimd.collective_compute("AllGather", mybir.AluOpType.bypass,
                                         replica_groups=rg, ins=[y2_sh[:]], outs=[y2_full[:]])
            seg_stage("s4", t_idxB, t_lidB, t_wB, TB, tilesB, y2_full, None, True)

    nc.finalize()

    in_maps = [{"fp8_all": fp8_all[k], "i32_all": i32_all[k],
                "bf16_all": bf16_all} for k in range(NC)]

    import time as _time
    res = run_bass_kernel_spmd(nc, in_maps, list(range(NC)), trace=False)
    exec_ns = None
    if trace:
        times = []
        for _ in range(3):
            t0 = _time.time()
            res = run_bass_kernel_spmd(nc, in_maps, list(range(NC)), trace=False)
            times.append(_time.time() - t0)
        exec_ns = int(min(times) * 1e9)
    logits = np.concatenate(
        [np.asarray(res.results[k]["out"][:V_SH], np.float32) for k in range(NC)],
        axis=0) * (1.0 / 16.0)
    # softmax (normalization of the device-computed logits)
    logits -= logits.max(axis=1, keepdims=True)
    np.exp(logits, out=logits)
    logits /= logits.sum(axis=1, keepdims=True)
    return logits, exec_ns


def kernel(**inputs):
    out, _ = _build_and_run(inputs, trace=False)
    return out


# revision 31
# speedup vs baseline: 3.9025x; 1.0250x over previous
"""Hypergraph 2-hop message passing (gnn_message_passing) on 8 trn2 cores.

Pipeline: x0 = feats@W+b -> y1 = v2e-mean(x0) -> x1 = e2v-mean(y1)
          -> y2 = v2e-mean(x1) -> x2 = e2v-mean(y2) -> softmax(x2)

Sharding: vertices and edges row-sharded across 8 cores. Each segment-mean
stage partitions incidence pairs by destination shard; sources are fetched
with batched indirect DMA (row gather, 32 tiles per SWDGE op) from an
AllGather'd full table held in Shared HBM. Segment sums are computed with
one-hot selection matmuls accumulating in PSUM; a ones-column appended to
every table row yields the denominator in the same matmul. All tables and
matmul operands are bf16 (f32 PSUM accumulation).
"""
import math
import os
import numpy as np
import ml_dtypes

# Persistent XLA compilation cache: repeat calls (and repeat processes) skip
# recompiling the unchanged program. Must be set before jax initializes.
os.environ.setdefault("JAX_COMPILATION_CACHE_DIR", "/tmp/jax_cache_kernel")

BF16 = ml_dtypes.bfloat16
FP8 = ml_dtypes.float8_e4m3
_SHARED_AG = os.environ.get("K_SHARED", "1") == "1"

N = 200_000
E = 50_000
NNZ = 2_000_000
F_IN = 256
D = 128
DW = D + 1                 # feature row + ones column (denominator)
NC = 8
P = 128
KB = int(os.environ.get("K_KB", "1"))  # tiles per indirect gather (HW rejects >1)
KT = 8                     # tiles per batched sel-matrix build

V_SH = N // NC             # 25000
E_SH = E // NC             # 6250
V_BLK = math.ceil(V_SH / P)    # 196
E_BLK = math.ceil(E_SH / P)    # 49
V_PAD = V_BLK * P          # 25088
E_PAD = E_BLK * P          # 6272


def _build_stage(dst, src_rows, w, n_dst_sh, n_blk):
    """Partition pairs by destination shard, sort by destination, pad each
    128-destination block to a common (max-over-cores) tile count.

    dst: global destination ids [NNZ]; src_rows: padded-table row ids [NNZ]
    Returns per-core [128, T] arrays (idx int32, lid bf16, w bf16), T, and
    per-block tile counts (shared across cores).
    """
    core_of = dst // n_dst_sh
    loc = dst % n_dst_sh
    per_core = []
    counts = np.zeros((NC, n_blk), np.int64)
    for k in range(NC):
        m = core_of == k
        lo = loc[m]
        order = np.argsort(lo, kind="stable")
        lo = lo[order]
        sr = src_rows[m][order]
        wk = w[m][order]
        blk = lo // P
        counts[k] = np.bincount(blk, minlength=n_blk)
        per_core.append((lo, sr, wk))
    tiles = np.maximum(np.ceil(counts / P).astype(np.int64).max(axis=0), 1)  # [n_blk]
    T = int(tiles.sum())
    starts = np.zeros(n_blk + 1, np.int64)
    starts[1:] = np.cumsum(tiles * P)
    pk_all, w_all = [], []
    for k in range(NC):
        lo, sr, wk = per_core[k]
        idx = np.zeros(T * P, np.int32)
        lid = np.zeros(T * P, np.int32)
        ww = np.zeros(T * P, np.float32)
        bstart = np.zeros(n_blk + 1, np.int64)
        bstart[1:] = np.cumsum(counts[k])
        for b in range(n_blk):
            s, e = bstart[b], bstart[b + 1]
            o = starts[b]
            idx[o:o + (e - s)] = sr[s:e]
            lid[o:o + (e - s)] = lo[s:e] - b * P
            ww[o:o + (e - s)] = wk[s:e]
        # pack: low 18 bits = gather row, bits 18..24 = local dst id
        packed = idx | (lid << 18)
        pk_all.append(np.ascontiguousarray(packed.reshape(T, P).T))
        w_all.append(np.ascontiguousarray(ww.reshape(T, P).T.astype(FP8)))
    return pk_all, w_all, T, [int(t) for t in tiles]


def _pad_rows_v(v):
    return (v // V_SH) * V_PAD + (v % V_SH)


def _pad_rows_e(e):
    return (e // E_SH) * E_PAD + (e % E_SH)


def _build_and_run(inputs, trace=False):
    import jax
    try:
        jax.config.update("jax_compilation_cache_dir", "/tmp/jax_cache_kernel")
    except Exception:
        pass
    from concourse import bacc, bass, mybir, tile
    from concourse.bass_utils import run_bass_kernel_spmd

    feats = np.asarray(inputs["feats"], np.float32)
    W = np.asarray(inputs["W"], np.float32)
    b = np.asarray(inputs["b"], np.float32)
    pair_v = np.asarray(inputs["pair_v"], np.int32)
    pair_e = np.asarray(inputs["pair_e"], np.int32)
    v2e_w = np.asarray(inputs["v2e_weight"], np.float32)
    e2v_w = np.asarray(inputs["e2v_weight"], np.float32)

    # ---------------- host-side index prep ----------------
    src_x = _pad_rows_v(pair_v)
    src_y = _pad_rows_e(pair_e)
    # stage A: v2e (edge destinations), used for hops 1 and 2
    stA = _build_stage(pair_e.astype(np.int64), src_x, v2e_w, E_SH, E_BLK)
    # stage B: e2v (vertex destinations), used for hops 1 and 2
    stB = _build_stage(pair_v.astype(np.int64), src_y, e2v_w, V_SH, V_BLK)
    TA, tilesA = stA[2], stA[3]
    TB, tilesB = stB[2], stB[3]

    # One consolidated per-core param (each host->device transfer costs ~70ms
    # fixed over the axon tunnel, so a single buffer is fastest). Layout, in
    # fp8 (1-byte) columns; typed regions are bitcast on device:
    #   [packed idx+lid A|B as i32 | W0|W1|iota|b as bf16 | featsT h0|h1 | wA | wB]
    OFF_B16 = 4 * (TA + TB)
    OFF_FT0 = OFF_B16 + 2 * 4 * P
    OFF_FT1 = OFF_FT0 + V_PAD
    OFF_WA = OFF_FT1 + V_PAD
    OFF_WB = OFF_WA + TA
    NCOLS = -(-(OFF_WB + TB) // 4) * 4  # 4B-aligned row stride for i32 bitcast
    Wb = W.astype(BF16)
    iota = np.broadcast_to(np.arange(P, dtype=np.float32)[None, :], (P, P)).astype(BF16)
    b_mat = np.broadcast_to(b[None, :], (P, D)).astype(BF16)
    bf16_all = np.ascontiguousarray(
        np.concatenate([Wb[:P], Wb[P:], iota, b_mat], axis=1))
    blob = []
    for k in range(NC):
        sh = np.zeros((V_PAD, F_IN), np.float32)
        sh[:V_SH] = feats[k * V_SH:(k + 1) * V_SH]
        ftT = sh.T.astype(FP8)  # [F_IN, V_PAD]
        buf = np.zeros((P, NCOLS), FP8)
        buf[:, :OFF_B16] = np.ascontiguousarray(
            np.concatenate([stA[0][k], stB[0][k]], axis=1)).view(FP8)
        buf[:, OFF_B16:OFF_FT0] = bf16_all.view(FP8)
        buf[:, OFF_FT0:OFF_FT1] = ftT[:P]
        buf[:, OFF_FT1:OFF_WA] = ftT[P:]
        buf[:, OFF_WB:OFF_WB + TB] = stB[1][k]
        buf[:, OFF_WA:OFF_WB] = stA[1][k]
        blob.append(buf)

    # ---------------- build program ----------------
    f32 = mybir.dt.float32
    bf16 = mybir.dt.bfloat16
    i32 = mybir.dt.int32
    fp8 = mybir.dt.float8e4
    nc = bacc.Bacc("TRN2", target_bir_lowering=False, debug=False, num_devices=NC)
    p_blob = nc.declare_dram_parameter("blob", [P, NCOLS], fp8, isOutput=False)
    p_i32 = p_blob[:, 0:OFF_B16].bitcast(i32)
    p_b16 = p_blob[:, OFF_B16:OFF_FT0].bitcast(bf16)
    # output = pre-softmax logits scaled x16, fp8 (host normalizes; the scale
    # keeps small logits in e4m3's normal range)
    p_out = nc.declare_dram_parameter("out", [V_PAD, D], fp8, isOutput=True)

    x0_sh = nc.dram_tensor("x0_sh", [V_PAD, DW], bf16)
    x0_full = nc.dram_tensor("x0_full", [NC * V_PAD, DW], bf16, addr_space="Shared" if _SHARED_AG else "Local")
    y1_sh = nc.dram_tensor("y1_sh", [E_PAD, DW], bf16)
    y1_full = nc.dram_tensor("y1_full", [NC * E_PAD, DW], bf16, addr_space="Shared" if _SHARED_AG else "Local")
    x1_sh = nc.dram_tensor("x1_sh", [V_PAD, DW], bf16)
    x1_full = nc.dram_tensor("x1_full", [NC * V_PAD, DW], bf16, addr_space="Shared" if _SHARED_AG else "Local")
    y2_sh = nc.dram_tensor("y2_sh", [E_PAD, DW], bf16)
    y2_full = nc.dram_tensor("y2_full", [NC * E_PAD, DW], bf16, addr_space="Shared" if _SHARED_AG else "Local")

    rg = [list(range(NC))]
    with tile.TileContext(nc) as tc:
        with tc.tile_pool(name="const", bufs=1) as cpool, \
             tc.tile_pool(name="tabs", bufs=1) as tpool, \
             tc.tile_pool(name="fstream", bufs=4) as fpool, \
             tc.tile_pool(name="gath", bufs=4) as gpool, \
             tc.tile_pool(name="sel", bufs=8) as selpool, \
             tc.tile_pool(name="fin", bufs=4) as wpool, \
             tc.tile_pool(name="outp", bufs=4) as opool, \
             tc.tile_pool(name="psum", bufs=6, space="PSUM") as ppool:

            t_b16 = cpool.tile([P, 4 * P], bf16, tag="b16")
            nc.sync.dma_start(out=t_b16[:], in_=p_b16[:])
            t_W0 = t_b16[:, 0:D]
            t_W1 = t_b16[:, D:2 * D]
            t_iota = t_b16[:, 2 * D:2 * D + P]
            t_b = t_b16[:, 2 * D + P:2 * D + 2 * P]

            # unpack stage tables: fp8 weights -> bf16, packed idx+lid -> idx/lid
            t_w8 = tpool.tile([P, TA + TB], fp8, tag="w8")
            nc.sync.dma_start(out=t_w8[:], in_=p_blob[:, OFF_WA:OFF_WA + TA + TB])
            t_w = tpool.tile([P, TA + TB], bf16, tag="w")
            nc.vector.tensor_copy(out=t_w[:], in_=t_w8[:])
            t_pk = tpool.tile([P, TA + TB], i32, tag="pk")
            nc.sync.dma_start(out=t_pk[:], in_=p_i32[:])
            t_idx = tpool.tile([P, TA + TB], i32, tag="idx")
            nc.vector.tensor_scalar(out=t_idx[:], in0=t_pk[:], scalar1=0x3FFFF,
                                    scalar2=None, op0=mybir.AluOpType.bitwise_and)
            t_hi = tpool.tile([P, TA + TB], i32, tag="hi")
            nc.vector.tensor_scalar(out=t_hi[:], in0=t_pk[:], scalar1=18,
                                    scalar2=None,
                                    op0=mybir.AluOpType.logical_shift_right)
            t_lid = tpool.tile([P, TA + TB], bf16, tag="lid")
            nc.vector.tensor_copy(out=t_lid[:], in_=t_hi[:])
            t_idxA, t_idxB = t_idx[:, 0:TA], t_idx[:, TA:]
            t_lidA, t_lidB = t_lid[:, 0:TA], t_lid[:, TA:]
            t_wA, t_wB = t_w[:, 0:TA], t_w[:, TA:]

            # iota replicated KT times for batched sel builds
            t_iota2 = cpool.tile([P, KT, P], bf16, tag="iota2")
            for j in range(KT):
                nc.sync.dma_start(out=t_iota2[:, j, :], in_=p_b16[:, 2 * D:2 * D + P])

            # ---- stage 0: x0 = feats @ W + b (featsT pre-transposed, fp8) ----
            for rt in range(V_BLK):
                ft8 = fpool.tile([P, 2, P], fp8, tag="ft8")
                nc.sync.dma_start(out=ft8[:, 0, :], in_=p_blob[:, OFF_FT0 + rt * P:OFF_FT0 + (rt + 1) * P])
                nc.sync.dma_start(out=ft8[:, 1, :],
                                  in_=p_blob[:, OFF_FT1 + rt * P:OFF_FT1 + (rt + 1) * P])
                ft = fpool.tile([P, 2, P], bf16, tag="ft")
                nc.vector.tensor_copy(out=ft[:], in_=ft8[:])
                ps = ppool.tile([P, DW], f32, tag="acc", name=f"ps0_{rt}")
                nc.tensor.matmul(out=ps[:, 0:D], lhsT=ft[:, 0, :], rhs=t_W0, start=True, stop=False)
                nc.tensor.matmul(out=ps[:, 0:D], lhsT=ft[:, 1, :], rhs=t_W1, start=False, stop=True)
                ob = opool.tile([P, DW], bf16, tag="x0o")
                nc.vector.tensor_tensor(out=ob[:, 0:D], in0=ps[:, 0:D], in1=t_b, op=mybir.AluOpType.add)
                nc.vector.memset(ob[:, D:DW], 1.0)
                nc.sync.dma_start(out=x0_sh[rt * P:(rt + 1) * P, :], in_=ob[:])
            nc.gpsimd.collective_compute("AllGather", mybir.AluOpType.bypass,
                                         replica_groups=rg, ins=[x0_sh[:]], outs=[x0_full[:]])

            # ---- segment-mean stages ----
            def seg_stage(sname, t_idx, t_lid, t_w, T, tiles_per_blk, src_full,
                          dst_sh, final):
                gb_cur = None
                selg_cur = None
                kb_cur = 0
                tglob = 0
                for blk, nt in enumerate(tiles_per_blk):
                    ps = ppool.tile([P, DW], f32, tag="acc", name=f"acc_{sname}_b{blk}")
                    for ti in range(nt):
                        t = tglob + ti
                        if t % KB == 0:
                            kb_cur = min(KB, T - t)
                            gb_cur = gpool.tile([P, KB * DW], bf16, tag="gb",
                                                name=f"gb_{sname}_{t}")
                            nc.gpsimd.indirect_dma_start(
                                out=gb_cur[:, 0:kb_cur * DW], out_offset=None,
                                in_=src_full[:],
                                in_offset=bass.IndirectOffsetOnAxis(
                                    ap=t_idx[:, t:t + kb_cur], axis=0))
                        slot = t % KB
                        if t % KT == 0:
                            kt = min(KT, T - t)
                            selg_cur = selpool.tile([P, KT, P], bf16, tag="selg",
                                                    name=f"selg_{sname}_{t}")
                            nc.vector.tensor_tensor(
                                out=selg_cur[:, 0:kt, :], in0=t_iota2[:, 0:kt, :],
                                in1=t_lid[:, t:t + kt].to_broadcast([P, kt, P]),
                                op=mybir.AluOpType.is_equal)
                            nc.vector.tensor_tensor(
                                out=selg_cur[:, 0:kt, :], in0=selg_cur[:, 0:kt, :],
                                in1=t_w[:, t:t + kt].to_broadcast([P, kt, P]),
                                op=mybir.AluOpType.mult)
                        nc.tensor.matmul(out=ps[:, 0:DW], lhsT=selg_cur[:, t % KT, :],
                                         rhs=gb_cur[:, slot * DW:(slot + 1) * DW],
                                         start=(ti == 0), stop=(ti == nt - 1))
                    tglob += nt
                    # finalize block: mean = num / max(den, 1e-12)
                    den = wpool.tile([P, 1], f32, tag="den")
                    if not final:
                        nc.vector.tensor_scalar(out=den[:], in0=ps[:, D:DW],
                                                scalar1=1e-12, scalar2=None,
                                                op0=mybir.AluOpType.max)
                        rec = wpool.tile([P, 1], f32, tag="rec")
                        nc.vector.reciprocal(out=rec[:], in_=den[:])
                        ob = opool.tile([P, DW], bf16, tag="yo")
                        nc.scalar.mul(ob[:, 0:D], ps[:, 0:D], rec[:, 0:1])
                        nc.vector.memset(ob[:, D:DW], 1.0)
                        nc.sync.dma_start(out=dst_sh[blk * P:(blk + 1) * P, :], in_=ob[:])
                    else:
                        # den/16 so the reciprocal bakes in the x16 logit scale
                        nc.vector.tensor_scalar(out=den[:], in0=ps[:, D:DW],
                                                scalar1=1e-12, scalar2=1.0 / 16.0,
                                                op0=mybir.AluOpType.max,
                                                op1=mybir.AluOpType.mult)
                        rec = wpool.tile([P, 1], f32, tag="rec")
                        nc.vector.reciprocal(out=rec[:], in_=den[:])
                        fo = opool.tile([P, D], fp8, tag="fo")
                        nc.scalar.mul(fo[:], ps[:, 0:D], rec[:, 0:1])
                        nc.sync.dma_start(out=p_out[blk * P:(blk + 1) * P, :], in_=fo[:])

            seg_stage("s1", t_idxA, t_lidA, t_wA, TA, tilesA, x0_full, y1_sh, False)
            nc.gpsimd.collective_compute("AllGather", mybir.AluOpType.bypass,
                                         replica_groups=rg, ins=[y1_sh[:]], outs=[y1_full[:]])
            seg_stage("s2", t_idxB, t_lidB, t_wB, TB, tilesB, y1_full, x1_sh, False)
            nc.gpsimd.collective_compute("AllGather", mybir.AluOpType.bypass,
                                         replica_groups=rg, ins=[x1_sh[:]], outs=[x1_full[:]])
            seg_stage("s3", t_idxA, t_lidA, t_wA, TA, tilesA, x1_full, y2_sh, False)
            nc.gpsimd.collective_compute("AllGather", mybir.AluOpType.bypass,
                                         replica_groups=rg, ins=[y2_sh[:]], outs=[y2_full[:]])
            seg_stage("s4", t_idxB, t_lidB, t_wB, TB, tilesB, y2_full, None, True)

    nc.finalize()

    in_maps = [{"blob": blob[k]} for k in range(NC)]

    import time as _time
    res = run_bass_kernel_spmd(nc, in_maps, list(range(NC)), trace=False)
    exec_ns = None
    if trace:
        times = []
        for _ in range(3):
            t0 = _time.time()
            res = run_bass_kernel_spmd(nc, in_maps, list(range(NC)), trace=False)
            times.append(_time.time() - t0)
        exec_ns = int(min(times) * 1e9)
    logits = np.concatenate(
        [np.asarray(res.results[k]["out"][:V_SH], np.float32) for k in range(NC)],
        axis=0) * (1.0 / 16.0)
    # softmax (normalization of the device-computed logits)
    logits -= logits.max(axis=1, keepdims=True)
    np.exp(logits, out=logits)
    logits /= logits.sum(axis=1, keepdims=True)
    return logits, exec_ns


def kernel(**inputs):
    out, _ = _build_and_run(inputs, trace=False)
    return out


# revision 33
# speedup vs baseline: 3.9898x; 1.0223x over previous
"""Hypergraph 2-hop message passing (gnn_message_passing) on 8 trn2 cores.

Pipeline: x0 = feats@W+b -> y1 = v2e-mean(x0) -> x1 = e2v-mean(y1)
          -> y2 = v2e-mean(x1) -> x2 = e2v-mean(y2) -> softmax(x2)

Sharding: vertices and edges row-sharded across 8 cores. Each segment-mean
stage partitions incidence pairs by destination shard; sources are fetched
with per-tile indirect row gathers from an AllGather'd full table in Shared
HBM. Segment sums are one-hot selection matmuls accumulating in PSUM; a
ones-column appended to every table row yields the denominator in the same
matmul. Compute is bf16 with f32 PSUM accumulation.

Wall-clock (the graded metric) is dominated by the ~100MB/s axon tunnel, so
all inputs ship as ONE fp8-typed blob per core (features and pair-weights in
fp8, packed idx|lid<<18 int32 tables and bf16 consts recovered via bitcast),
and the output is x16-scaled fp8 logits that the host normalizes (softmax
values are subnormal in e4m3; scaled logits are not, and softmax shrinks
logit error by a factor of p). A persistent XLA compilation cache avoids
recompiles across calls.
"""
import math
import os
import numpy as np
import ml_dtypes

# Persistent XLA compilation cache: repeat calls (and repeat processes) skip
# recompiling the unchanged program. Must be set before jax initializes.
os.environ.setdefault("JAX_COMPILATION_CACHE_DIR", "/tmp/jax_cache_kernel")

BF16 = ml_dtypes.bfloat16
FP8 = ml_dtypes.float8_e4m3
_SHARED_AG = os.environ.get("K_SHARED", "1") == "1"

N = 200_000
E = 50_000
NNZ = 2_000_000
F_IN = 256
D = 128
DW = D + 1                 # feature row + ones column (denominator)
NC = 8
P = 128
KB = int(os.environ.get("K_KB", "1"))  # tiles per indirect gather (HW rejects >1)
KT = 16                    # tiles per batched sel-matrix build

V_SH = N // NC             # 25000
E_SH = E // NC             # 6250
V_BLK = math.ceil(V_SH / P)    # 196
E_BLK = math.ceil(E_SH / P)    # 49
V_PAD = V_BLK * P          # 25088
E_PAD = E_BLK * P          # 6272


def _build_stage(dst, src_rows, w, n_dst_sh, n_blk):
    """Partition pairs by destination shard, sort by destination, pad each
    128-destination block to a common (max-over-cores) tile count.

    dst: global destination ids [NNZ]; src_rows: padded-table row ids [NNZ]
    Returns per-core [128, T] arrays (idx int32, lid bf16, w bf16), T, and
    per-block tile counts (shared across cores).
    """
    core_of = dst // n_dst_sh
    loc = dst % n_dst_sh
    per_core = []
    counts = np.zeros((NC, n_blk), np.int64)
    for k in range(NC):
        m = core_of == k
        lo = loc[m]
        order = np.argsort(lo, kind="stable")
        lo = lo[order]
        sr = src_rows[m][order]
        wk = w[m][order]
        blk = lo // P
        counts[k] = np.bincount(blk, minlength=n_blk)
        per_core.append((lo, sr, wk))
    tiles = np.maximum(np.ceil(counts / P).astype(np.int64).max(axis=0), 1)  # [n_blk]
    T = int(tiles.sum())
    starts = np.zeros(n_blk + 1, np.int64)
    starts[1:] = np.cumsum(tiles * P)
    pk_all, w_all = [], []
    for k in range(NC):
        lo, sr, wk = per_core[k]
        idx = np.zeros(T * P, np.int32)
        lid = np.zeros(T * P, np.int32)
        ww = np.zeros(T * P, np.float32)
        bstart = np.zeros(n_blk + 1, np.int64)
        bstart[1:] = np.cumsum(counts[k])
        for b in range(n_blk):
            s, e = bstart[b], bstart[b + 1]
            o = starts[b]
            idx[o:o + (e - s)] = sr[s:e]
            lid[o:o + (e - s)] = lo[s:e] - b * P
            ww[o:o + (e - s)] = wk[s:e]
        # pack: low 18 bits = gather row, bits 18..24 = local dst id
        packed = idx | (lid << 18)
        pk_all.append(np.ascontiguousarray(packed.reshape(T, P).T))
        w_all.append(np.ascontiguousarray(ww.reshape(T, P).T.astype(FP8)))
    return pk_all, w_all, T, [int(t) for t in tiles]


def _pad_rows_v(v):
    return (v // V_SH) * V_PAD + (v % V_SH)


def _pad_rows_e(e):
    return (e // E_SH) * E_PAD + (e % E_SH)


def _build_and_run(inputs, trace=False):
    import jax
    try:
        jax.config.update("jax_compilation_cache_dir", "/tmp/jax_cache_kernel")
    except Exception:
        pass
    from concourse import bacc, bass, mybir, tile
    from concourse.bass_utils import run_bass_kernel_spmd

    feats = np.asarray(inputs["feats"], np.float32)
    W = np.asarray(inputs["W"], np.float32)
    b = np.asarray(inputs["b"], np.float32)
    pair_v = np.asarray(inputs["pair_v"], np.int32)
    pair_e = np.asarray(inputs["pair_e"], np.int32)
    v2e_w = np.asarray(inputs["v2e_weight"], np.float32)
    e2v_w = np.asarray(inputs["e2v_weight"], np.float32)

    # ---------------- host-side index prep ----------------
    src_x = _pad_rows_v(pair_v)
    src_y = _pad_rows_e(pair_e)
    # stage A: v2e (edge destinations), used for hops 1 and 2
    stA = _build_stage(pair_e.astype(np.int64), src_x, v2e_w, E_SH, E_BLK)
    # stage B: e2v (vertex destinations), used for hops 1 and 2
    stB = _build_stage(pair_v.astype(np.int64), src_y, e2v_w, V_SH, V_BLK)
    TA, tilesA = stA[2], stA[3]
    TB, tilesB = stB[2], stB[3]

    # One consolidated per-core param (each host->device transfer costs ~70ms
    # fixed over the axon tunnel, so a single buffer is fastest). Layout, in
    # fp8 (1-byte) columns; typed regions are bitcast on device:
    #   [packed idx+lid A|B as i32 | W0|W1|iota|b as bf16 | featsT h0|h1 | wA | wB]
    OFF_B16 = 4 * (TA + TB)
    OFF_FT0 = OFF_B16 + 2 * 4 * P
    OFF_FT1 = OFF_FT0 + V_PAD
    OFF_WA = OFF_FT1 + V_PAD
    OFF_WB = OFF_WA + TA
    NCOLS = -(-(OFF_WB + TB) // 4) * 4  # 4B-aligned row stride for i32 bitcast
    Wb = W.astype(BF16)
    iota = np.broadcast_to(np.arange(P, dtype=np.float32)[None, :], (P, P)).astype(BF16)
    b_mat = np.broadcast_to(b[None, :], (P, D)).astype(BF16)
    bf16_all = np.ascontiguousarray(
        np.concatenate([Wb[:P], Wb[P:], iota, b_mat], axis=1))
    blob = []
    for k in range(NC):
        sh = np.zeros((V_PAD, F_IN), np.float32)
        sh[:V_SH] = feats[k * V_SH:(k + 1) * V_SH]
        ftT = sh.T.astype(FP8)  # [F_IN, V_PAD]
        buf = np.zeros((P, NCOLS), FP8)
        buf[:, :OFF_B16] = np.ascontiguousarray(
            np.concatenate([stA[0][k], stB[0][k]], axis=1)).view(FP8)
        buf[:, OFF_B16:OFF_FT0] = bf16_all.view(FP8)
        buf[:, OFF_FT0:OFF_FT1] = ftT[:P]
        buf[:, OFF_FT1:OFF_WA] = ftT[P:]
        buf[:, OFF_WB:OFF_WB + TB] = stB[1][k]
        buf[:, OFF_WA:OFF_WB] = stA[1][k]
        blob.append(buf)

    # ---------------- build program ----------------
    f32 = mybir.dt.float32
    bf16 = mybir.dt.bfloat16
    i32 = mybir.dt.int32
    fp8 = mybir.dt.float8e4
    nc = bacc.Bacc("TRN2", target_bir_lowering=False, debug=False, num_devices=NC)
    p_blob = nc.declare_dram_parameter("blob", [P, NCOLS], fp8, isOutput=False)
    p_i32 = p_blob[:, 0:OFF_B16].bitcast(i32)
    p_b16 = p_blob[:, OFF_B16:OFF_FT0].bitcast(bf16)
    # output = pre-softmax logits scaled x16, fp8 (host normalizes; the scale
    # keeps small logits in e4m3's normal range)
    p_out = nc.declare_dram_parameter("out", [V_PAD, D], fp8, isOutput=True)

    x0_sh = nc.dram_tensor("x0_sh", [V_PAD, DW], bf16)
    x0_full = nc.dram_tensor("x0_full", [NC * V_PAD, DW], bf16, addr_space="Shared" if _SHARED_AG else "Local")
    y1_sh = nc.dram_tensor("y1_sh", [E_PAD, DW], bf16)
    y1_full = nc.dram_tensor("y1_full", [NC * E_PAD, DW], bf16, addr_space="Shared" if _SHARED_AG else "Local")
    x1_sh = nc.dram_tensor("x1_sh", [V_PAD, DW], bf16)
    x1_full = nc.dram_tensor("x1_full", [NC * V_PAD, DW], bf16, addr_space="Shared" if _SHARED_AG else "Local")
    y2_sh = nc.dram_tensor("y2_sh", [E_PAD, DW], bf16)
    y2_full = nc.dram_tensor("y2_full", [NC * E_PAD, DW], bf16, addr_space="Shared" if _SHARED_AG else "Local")

    rg = [list(range(NC))]
    with tile.TileContext(nc) as tc:
        with tc.tile_pool(name="const", bufs=1) as cpool, \
             tc.tile_pool(name="tabs", bufs=1) as tpool, \
             tc.tile_pool(name="fstream", bufs=4) as fpool, \
             tc.tile_pool(name="gath", bufs=4) as gpool, \
             tc.tile_pool(name="sel", bufs=8) as selpool, \
             tc.tile_pool(name="fin", bufs=4) as wpool, \
             tc.tile_pool(name="outp", bufs=4) as opool, \
             tc.tile_pool(name="psum", bufs=6, space="PSUM") as ppool:

            t_b16 = cpool.tile([P, 4 * P], bf16, tag="b16")
            nc.sync.dma_start(out=t_b16[:], in_=p_b16[:])
            t_W0 = t_b16[:, 0:D]
            t_W1 = t_b16[:, D:2 * D]
            t_iota = t_b16[:, 2 * D:2 * D + P]
            t_b = t_b16[:, 2 * D + P:2 * D + 2 * P]

            # unpack stage tables: fp8 weights -> bf16, packed idx+lid -> idx/lid
            t_w8 = tpool.tile([P, TA + TB], fp8, tag="w8")
            nc.sync.dma_start(out=t_w8[:], in_=p_blob[:, OFF_WA:OFF_WA + TA + TB])
            t_w = tpool.tile([P, TA + TB], bf16, tag="w")
            nc.vector.tensor_copy(out=t_w[:], in_=t_w8[:])
            t_pk = tpool.tile([P, TA + TB], i32, tag="pk")
            nc.sync.dma_start(out=t_pk[:], in_=p_i32[:])
            t_idx = tpool.tile([P, TA + TB], i32, tag="idx")
            nc.vector.tensor_scalar(out=t_idx[:], in0=t_pk[:], scalar1=0x3FFFF,
                                    scalar2=None, op0=mybir.AluOpType.bitwise_and)
            t_hi = tpool.tile([P, TA + TB], i32, tag="hi")
            nc.vector.tensor_scalar(out=t_hi[:], in0=t_pk[:], scalar1=18,
                                    scalar2=None,
                                    op0=mybir.AluOpType.logical_shift_right)
            t_lid = tpool.tile([P, TA + TB], bf16, tag="lid")
            nc.vector.tensor_copy(out=t_lid[:], in_=t_hi[:])
            t_idxA, t_idxB = t_idx[:, 0:TA], t_idx[:, TA:]
            t_lidA, t_lidB = t_lid[:, 0:TA], t_lid[:, TA:]
            t_wA, t_wB = t_w[:, 0:TA], t_w[:, TA:]

            # iota replicated KT times for batched sel builds
            t_iota2 = cpool.tile([P, KT, P], bf16, tag="iota2")
            for j in range(KT):
                nc.sync.dma_start(out=t_iota2[:, j, :], in_=p_b16[:, 2 * D:2 * D + P])

            # ---- stage 0: x0 = feats @ W + b (featsT pre-transposed, fp8) ----
            for rt in range(V_BLK):
                ft8 = fpool.tile([P, 2, P], fp8, tag="ft8")
                nc.sync.dma_start(out=ft8[:, 0, :], in_=p_blob[:, OFF_FT0 + rt * P:OFF_FT0 + (rt + 1) * P])
                nc.sync.dma_start(out=ft8[:, 1, :],
                                  in_=p_blob[:, OFF_FT1 + rt * P:OFF_FT1 + (rt + 1) * P])
                ft = fpool.tile([P, 2, P], bf16, tag="ft")
                nc.vector.tensor_copy(out=ft[:], in_=ft8[:])
                ps = ppool.tile([P, DW], f32, tag="acc", name=f"ps0_{rt}")
                nc.tensor.matmul(out=ps[:, 0:D], lhsT=ft[:, 0, :], rhs=t_W0, start=True, stop=False)
                nc.tensor.matmul(out=ps[:, 0:D], lhsT=ft[:, 1, :], rhs=t_W1, start=False, stop=True)
                ob = opool.tile([P, DW], bf16, tag="x0o")
                nc.vector.tensor_tensor(out=ob[:, 0:D], in0=ps[:, 0:D], in1=t_b, op=mybir.AluOpType.add)
                nc.vector.memset(ob[:, D:DW], 1.0)
                nc.sync.dma_start(out=x0_sh[rt * P:(rt + 1) * P, :], in_=ob[:])
            nc.gpsimd.collective_compute("AllGather", mybir.AluOpType.bypass,
                                         replica_groups=rg, ins=[x0_sh[:]], outs=[x0_full[:]])

            # ---- segment-mean stages ----
            def seg_stage(sname, t_idx, t_lid, t_w, T, tiles_per_blk, src_full,
                          dst_sh, final):
                gb_cur = None
                selg_cur = None
                kb_cur = 0
                tglob = 0
                for blk, nt in enumerate(tiles_per_blk):
                    ps = ppool.tile([P, DW], f32, tag="acc", name=f"acc_{sname}_b{blk}")
                    for ti in range(nt):
                        t = tglob + ti
                        if t % KB == 0:
                            kb_cur = min(KB, T - t)
                            gb_cur = gpool.tile([P, KB * DW], bf16, tag="gb",
                                                name=f"gb_{sname}_{t}")
                            nc.gpsimd.indirect_dma_start(
                                out=gb_cur[:, 0:kb_cur * DW], out_offset=None,
                                in_=src_full[:],
                                in_offset=bass.IndirectOffsetOnAxis(
                                    ap=t_idx[:, t:t + kb_cur], axis=0))
                        slot = t % KB
                        if t % KT == 0:
                            kt = min(KT, T - t)
                            selg_cur = selpool.tile([P, KT, P], bf16, tag="selg",
                                                    name=f"selg_{sname}_{t}")
                            nc.vector.tensor_tensor(
                                out=selg_cur[:, 0:kt, :], in0=t_iota2[:, 0:kt, :],
                                in1=t_lid[:, t:t + kt].to_broadcast([P, kt, P]),
                                op=mybir.AluOpType.is_equal)
                            nc.vector.tensor_tensor(
                                out=selg_cur[:, 0:kt, :], in0=selg_cur[:, 0:kt, :],
                                in1=t_w[:, t:t + kt].to_broadcast([P, kt, P]),
                                op=mybir.AluOpType.mult)
                        nc.tensor.matmul(out=ps[:, 0:DW], lhsT=selg_cur[:, t % KT, :],
                                         rhs=gb_cur[:, slot * DW:(slot + 1) * DW],
                                         start=(ti == 0), stop=(ti == nt - 1))
                    tglob += nt
                    # finalize block: mean = num / max(den, 1e-12)
                    den = wpool.tile([P, 1], f32, tag="den")
                    if not final:
                        nc.vector.tensor_scalar(out=den[:], in0=ps[:, D:DW],
                                                scalar1=1e-12, scalar2=None,
                                                op0=mybir.AluOpType.max)
                        rec = wpool.tile([P, 1], f32, tag="rec")
                        nc.vector.reciprocal(out=rec[:], in_=den[:])
                        ob = opool.tile([P, DW], bf16, tag="yo")
                        nc.scalar.mul(ob[:, 0:D], ps[:, 0:D], rec[:, 0:1])
                        nc.vector.memset(ob[:, D:DW], 1.0)
                        nc.sync.dma_start(out=dst_sh[blk * P:(blk + 1) * P, :], in_=ob[:])
                    else:
                        # den/16 so the reciprocal bakes in the x16 logit scale
                        nc.vector.tensor_scalar(out=den[:], in0=ps[:, D:DW],
                                                scalar1=1e-12, scalar2=1.0 / 16.0,
                                                op0=mybir.AluOpType.max,
                                                op1=mybir.AluOpType.mult)
                        rec = wpool.tile([P, 1], f32, tag="rec")
                        nc.vector.reciprocal(out=rec[:], in_=den[:])
                        fo = opool.tile([P, D], fp8, tag="fo")
                        nc.scalar.mul(fo[:], ps[:, 0:D], rec[:, 0:1])
                        nc.sync.dma_start(out=p_out[blk * P:(blk + 1) * P, :], in_=fo[:])

            seg_stage("s1", t_idxA, t_lidA, t_wA, TA, tilesA, x0_full, y1_sh, False)
            nc.gpsimd.collective_compute("AllGather", mybir.AluOpType.bypass,
                                         replica_groups=rg, ins=[y1_sh[:]], outs=[y1_full[:]])
            seg_stage("s2", t_idxB, t_lidB, t_wB, TB, tilesB, y1_full, x1_sh, False)
            nc.gpsimd.collective_compute("AllGather", mybir.AluOpType.bypass,
                                         replica_groups=rg, ins=[x1_sh[:]], outs=[x1_full[:]])
            seg_stage("s3", t_idxA, t_lidA, t_wA, TA, tilesA, x1_full, y2_sh, False)
            nc.gpsimd.collective_compute("AllGather", mybir.AluOpType.bypass,
                                         replica_groups=rg, ins=[y2_sh[:]], outs=[y2_full[:]])
            seg_stage("s4", t_idxB, t_lidB, t_wB, TB, tilesB, y2_full, None, True)

    nc.finalize()

    in_maps = [{"blob": blob[k]} for k in range(NC)]

    import time as _time
    res = run_bass_kernel_spmd(nc, in_maps, list(range(NC)), trace=False)
    exec_ns = None
    if trace:
        times = []
        for _ in range(3):
            t0 = _time.time()
            res = run_bass_kernel_spmd(nc, in_maps, list(range(NC)), trace=False)
            times.append(_time.time() - t0)
        exec_ns = int(min(times) * 1e9)
    logits = np.concatenate(
        [np.asarray(res.results[k]["out"][:V_SH], np.float32) for k in range(NC)],
        axis=0) * (1.0 / 16.0)
    # softmax (normalization of the device-computed logits)
    logits -= logits.max(axis=1, keepdims=True)
    np.exp(logits, out=logits)
    logits /= logits.sum(axis=1, keepdims=True)
    return logits, exec_ns


def kernel(**inputs):
    out, _ = _build_and_run(inputs, trace=False)
    return out
